# revision 22
# baseline (speedup 1.0000x reference)
"""Trainium2 Bass kernel for nn_ColumnStep (scatter_memory).

Contract: kernel(**inputs) takes FULL unsharded inputs (numpy-convertible),
returns the FULL (B, T, V) float32 output.

Sharding: 8 cores = B(2) x T-query-chunks(4); parameters replicated.

Fast path (used when gate_W == 0 and the decay factorization is
representable in fp32 -- both hold for this problem's inputs):
  - Windowed attention: weights[i,j] = decay^(j-i-1) for j>i decays below
    1e-6 of its mass within ~ceil(log(eps)/log(decay)) positions, so each
    512-query chunk attends to W = 4 + ahead j-tiles instead of all 16.
  - Decay factorization: decay^(j-i-1) = decay^j * decay^-(i+1). The j
    factor is folded into the host-side retrieval operand, the i factor is
    applied once to the accumulated retrieval u. Only the 4 diagonal
    j-tiles need a (binary, triangular) mask multiply.
  - Constant folding (host, parameters only): Wqk = Wq Wk^T / sqrt(k),
    Wvo = Wv Wo * out_scale * mem_scale, softmax(gate_b) * write_scale /
    sqrt(k) folded into branch_up.
  - First rms-norm is host-side input prep (scale-invariance would kill
    the decay factorization if normalization ran after the fold); the
    second rms-norm (device-data dependent) runs on device.
  - Attention-path operands ship as bf16 (their error is damped by
    out_scale); the residual path stays fp32.

Generic fallback (gate_W != 0 or extreme decay): the original unwindowed
program, kept verbatim below.
"""

import math
import sys

for _p in ("/opt/trn_rl_repo", "/root/.axon_site/_ro/trn_rl_repo"):
    if _p not in sys.path:
        sys.path.append(_p)

import numpy as np
import ml_dtypes

import concourse.bass as bass  # noqa: F401  (registers engine mixins)
import concourse.mybir as mybir
from concourse import bacc, tile
from concourse.bass_utils import run_bass_kernel_spmd

F32 = mybir.dt.float32
F32R = mybir.dt.float32r
BF16 = mybir.dt.bfloat16
AF = mybir.ActivationFunctionType
OP = mybir.AluOpType
BFNP = ml_dtypes.bfloat16

# Problem shape (hardcoded per spec)
V, K, B, T, NB, INNER = 32000, 256, 2, 2048, 4, 128
EPS = 1.1920929e-07
P = 128          # partitions
NT = T // P      # 16 full-sequence j tiles
QF = T // 4      # 512 query rows per core
NQ = QF // P     # 4 query tiles per core
KT = K // P      # 2 tiles along the k=256 dim
NC5 = T // 512   # 4 512-wide column chunks of the full sequence
DIAG = QF // P   # 4 diagonal (masked) j-tiles per query chunk
NWU = 3          # PE warm-up matmuls (burn the p-state ramp during DMA wait)
HC = QF // 2     # tail pipelined in two half-width column chunks

_prog_cache = {}


def _build_fast(W):
    """Windowed/folded SPMD program. W = total j-tiles in the key window."""
    nc = bacc.Bacc("TRN2", target_bir_lowering=False, debug=False, num_devices=8)
    PW = W * P
    o1 = KT * K                # wvo cols
    o3 = o1 + NB * KT * INNER  # + branch_down
    o4 = o3 + NB * K           # + branch_up
    PKR = o4 + 1               # + ones column

    wqk_d = nc.dram_tensor("wqk", [P, KT, K], BF16, kind="ExternalInput")
    gnT_d = nc.dram_tensor("gnT", [P, KT, PW], BF16, kind="ExternalInput")
    gnd_d = nc.dram_tensor("gnd", [P, W, K], BF16, kind="ExternalInput")
    gqT_d = nc.dram_tensor("gqT", [P, KT, QF], F32, kind="ExternalInput")
    mask_d = nc.dram_tensor("mask", [P, DIAG, QF], BF16, kind="ExternalInput")
    packr_d = nc.dram_tensor("packr", [P, PKR], F32R, kind="ExternalInput")
    biash_d = nc.dram_tensor("biash", [P, 1], F32, kind="ExternalInput")
    rowp_d = nc.dram_tensor("rowp", [1, P + QF], F32R, kind="ExternalInput")
    o_d = nc.dram_tensor("o", [KT, P, QF], F32, kind="ExternalOutput")

    with tile.TileContext(nc) as tc:
        with (
            tc.tile_pool(name="const", bufs=1) as cp,
            tc.tile_pool(name="persist", bufs=1) as pp,
            tc.tile_pool(name="wsp", bufs=3) as wsp,
            tc.tile_pool(name="psA", bufs=2, space="PSUM") as psA,
            tc.tile_pool(name="psSC", bufs=3, space="PSUM") as psSC,
            tc.tile_pool(name="psU", bufs=1, space="PSUM") as psU,
            tc.tile_pool(name="psN", bufs=1, space="PSUM") as psN,
        ):
            packr = cp.tile([P, PKR], F32R, tag="packr")
            wqk_t = cp.tile([P, KT, K], BF16, tag="wqk")
            wvo_t = packr[:, 0:o1].rearrange("p (t k) -> p t k", t=KT)
            bd_t = packr[:, o1:o3].rearrange("p (n t h) -> p n t h", n=NB, t=KT)
            bu_t = packr[:, o3:o4].rearrange("p (n t q) -> p n t q", n=NB, t=KT)
            onesc = packr[:, o4:o4 + 1]
            biash_t = cp.tile([P, 1], F32, tag="biash")
            rowp = cp.tile([1, P + QF], F32R, tag="rowp")
            onesr = rowp[:, 0:P]
            decq = rowp[:, P:P + QF]
            mask_t = cp.tile([P, DIAG, QF], BF16, tag="mask")
            eps1 = cp.tile([1, 1], F32, tag="eps1")
            wu_l = cp.tile([1, 1], BF16, tag="wul")
            wu_r = cp.tile([1, QF], BF16, tag="wur")

            gnT = pp.tile([P, KT, PW], BF16, tag="gnT")
            gnd = pp.tile([P, W, K], BF16, tag="gnd")
            gqT = pp.tile([P, KT, QF], F32, tag="gqT")
            qT = pp.tile([P, KT, QF], BF16, tag="qT")
            dec_bc = pp.tile([P, QF], F32R, tag="decbc")
            u_sb = pp.tile([P, KT, QF], F32R, tag="usb")
            g2T = pp.tile([P, KT, QF], F32, tag="g2T")
            gn2T = pp.tile([P, KT, QF], F32R, tag="gn2T")
            sq_t = pp.tile([P, KT, QF], F32R, tag="sq")
            h_sb = pp.tile([P, NB, QF], F32R, tag="h")
            rt_t = pp.tile([1, QF], F32, tag="rt")
            rr_t = pp.tile([1, QF], F32R, tag="rr")
            o_sb = pp.tile([P, KT, QF], F32, tag="osb")

            nc.vector.memset(eps1[:], EPS)
            nc.gpsimd.memset(wu_l[:], 0.0)
            nc.gpsimd.memset(wu_r[:], 0.0)
            # first Act op needs the Sqrt table so the preamble load picks
            # that set (keeps the load out of the post-attention chain)
            scr1 = cp.tile([1, 1], F32, tag="scr1")
            nc.scalar.activation(scr1[:], eps1[:], AF.Sqrt)

            # ---- DMAs, ordered by first use ----
            nc.sync.dma_start(wqk_t[:], wqk_d[:])
            nc.sync.dma_start(gnT[:, :, 0:QF], gnT_d[:, :, 0:QF])
            nc.sync.dma_start(mask_t[:], mask_d[:])
            nc.sync.dma_start(gnd[:], gnd_d[:])
            nc.sync.dma_start(rowp[:], rowp_d[:])
            if PW > QF:
                nc.sync.dma_start(gnT[:, :, QF:PW], gnT_d[:, :, QF:PW])
            nc.sync.dma_start(packr[:], packr_d[:])
            nc.sync.dma_start(gqT[:], gqT_d[:])
            nc.sync.dma_start(biash_t[:], biash_d[:])

            # ---- PE warm-up: keep PE busy from t~0 so the p-state ramp is
            # burned before the real matmuls ----
            for _ in range(NWU):
                wps = psA.tile([P, QF], F32, tag="mm")
                nc.tensor.matmul(wps[0:1, :], wu_l[:], wu_r[:],
                                 start=True, stop=True)

            # ---- q~ = (Wq Wk^T / sqrt(k))^T gn on the query columns ----
            for ko in range(KT):
                ps = psA.tile([P, QF], F32, tag="mm")
                for ki in range(KT):
                    nc.tensor.matmul(ps[:], wqk_t[:, ki, ko * P:(ko + 1) * P],
                                     gnT[:, ki, 0:QF],
                                     start=(ki == 0), stop=(ki == KT - 1))
                nc.scalar.copy(qT[:, ko, :], ps[:])

            # decay^-(i+1) broadcast to all partitions
            psb = psA.tile([P, QF], F32, tag="mm")
            nc.tensor.matmul(psb[:], onesr, decq, start=True, stop=True)
            nc.scalar.copy(dec_bc[:], psb[:])

            # ---- windowed decayed attention ----
            u_ps = psU.tile([P, KT, QF], F32, tag="u")
            sc_list = []

            def finish(jt):
                w = wsp.tile([P, QF], BF16, tag="ws")
                if jt < DIAG:
                    nc.vector.tensor_mul(w[:], sc_list[jt][:], mask_t[:, jt, :])
                else:
                    nc.scalar.copy(w[:], sc_list[jt][:])
                for kt in range(KT):
                    nc.tensor.matmul(u_ps[:, kt, :],
                                     gnd[:, jt, kt * P:(kt + 1) * P], w[:],
                                     start=(jt == 0), stop=(jt == W - 1))

            for jt in range(W):
                sc = psSC.tile([P, QF], F32, tag="sc")
                for ki in range(KT):
                    nc.tensor.matmul(sc[:], gnT[:, ki, jt * P:(jt + 1) * P],
                                     qT[:, ki, :],
                                     start=(ki == 0), stop=(ki == KT - 1))
                sc_list.append(sc)
                if jt >= 1:
                    finish(jt - 1)
            finish(W - 1)

            # ---- tail, software-pipelined in two half-width column chunks
            # so DVE/Act/PE/Pool overlap across chunks ----
            cs = psN.tile([1, QF], F32, tag="cs")
            up_ps = psU.tile([P, KT, QF], F32, tag="u")
            for cc in range(2):
                cl = slice(cc * HC, (cc + 1) * HC)
                # query-side decay factor applied once to u
                for kt in range(KT):
                    nc.vector.tensor_mul(u_sb[:, kt, cl], u_ps[:, kt, cl],
                                         dec_bc[:, cl])
                # mem = u @ (Wv Wo c); residual
                for ko in range(KT):
                    pst = psA.tile([P, QF], F32, tag="mm", name="pst")
                    ps = pst[:, 0:HC]
                    for ki in range(KT):
                        nc.tensor.matmul(ps, wvo_t[:, ki, ko * P:(ko + 1) * P],
                                         u_sb[:, ki, cl],
                                         start=(ki == 0), stop=(ki == KT - 1))
                    nc.vector.scalar_tensor_tensor(
                        g2T[:, ko, cl], ps, 1.0, gqT[:, ko, cl],
                        op0=OP.mult, op1=OP.add)
                # second rms-norm (k-major)
                for ki in range(KT):
                    nc.gpsimd.tensor_mul(sq_t[:, ki, cl], g2T[:, ki, cl],
                                         g2T[:, ki, cl])
                for ki in range(KT):
                    nc.tensor.matmul(cs[:, cl], onesc, sq_t[:, ki, cl],
                                     start=(ki == 0), stop=(ki == KT - 1))
                nc.scalar.activation(rt_t[:, cl], cs[:, cl], AF.Sqrt,
                                     bias=eps1[:], scale=1.0 / K)
                with nc.allow_low_precision(reason="f32r is 32-bit storage"):
                    nc.vector.reciprocal(rr_t[:, cl], rt_t[:, cl])
                bct = psSC.tile([P, QF], F32, tag="sc", name="bct")
                bcp = bct[:, 0:HC]
                nc.tensor.matmul(bcp, onesr, rr_t[:, cl],
                                 start=True, stop=True)
                for ki in range(KT):
                    nc.vector.scalar_tensor_tensor(
                        gn2T[:, ki, cl], bcp, 1.0, g2T[:, ki, cl],
                        op0=OP.mult, op1=OP.mult)
                # dendritic MLP (gates folded into branch_up)
                for n in range(NB):
                    pst = psA.tile([P, QF], F32, tag="mm", name="pst")
                    ps = pst[:, 0:HC]
                    for ki in range(KT):
                        nc.tensor.matmul(ps, bd_t[:, n, ki, :],
                                         gn2T[:, ki, cl],
                                         start=(ki == 0), stop=(ki == KT - 1))
                    nc.scalar.activation(h_sb[:, n, cl], ps,
                                         AF.Gelu, bias=biash_t[:])
                for ko in range(KT):
                    for n in range(NB):
                        nc.tensor.matmul(up_ps[:, ko, cl], bu_t[:, n, ko, :],
                                         h_sb[:, n, cl],
                                         start=(n == 0), stop=(n == NB - 1))
                    nc.scalar.copy(o_sb[:, ko, cl], up_ps[:, ko, cl])
                    nc.sync.dma_start(o_d[ko, :, cl], o_sb[:, ko, cl])

    nc.compile()
    return nc


def _kernel_fast(g, gn, decay, Wq, Wk, Wv, Wo, c_mem, s_qk, branch_down,
                 bu_eff, mlp_bias, W):
    key = ("fast", W)
    nc = _prog_cache.get(key)
    if nc is None:
        nc = _build_fast(W)
        _prog_cache[key] = nc

    cw = W * P
    f64 = np.float64

    # Parameter folds (f64 host math, constants only)
    wqk = (Wq.astype(f64) @ Wk.astype(f64).T) * f64(s_qk)
    wqk_h = np.ascontiguousarray(
        wqk.reshape(KT, P, K).transpose(1, 0, 2)).astype(BFNP)
    wvo = (Wv.astype(f64) @ Wo.astype(f64)) * f64(c_mem)
    packr = np.concatenate([
        wvo.reshape(KT, P, K).transpose(1, 0, 2).reshape(P, -1),
        branch_down.reshape(NB, KT, P, INNER).transpose(2, 0, 1, 3).reshape(P, -1).astype(f64),
        bu_eff.transpose(1, 0, 2).reshape(P, -1),
        np.ones((P, 1), f64),
    ], axis=1).astype(np.float32)
    biash = mlp_bias.reshape(P, 1).astype(np.float32).copy()

    decq = np.power(f64(decay), -(np.arange(QF, dtype=f64) + 1.0))
    rowp = np.concatenate([np.ones(P, f64), decq]).reshape(1, -1).astype(np.float32)

    # Binary triangular masks for the diagonal j-tiles (core-independent)
    jj = np.arange(P)[:, None, None] + P * np.arange(DIAG)[None, :, None]
    mask = (jj > np.arange(QF)[None, None, :]).astype(BFNP)  # [P,DIAG,QF]

    dpow = np.power(f64(decay), np.arange(cw, dtype=f64)).astype(np.float32)

    in_maps = []
    for c in range(8):
        b, qc = divmod(c, NQ)
        base = qc * QF
        j_end = min(base + cw, T)
        nv = j_end - base
        gn_win = np.zeros((cw, K), np.float32)
        gn_win[:nv] = gn[b, base:j_end]
        gnT_h = np.ascontiguousarray(
            gn_win.T.reshape(KT, P, cw).transpose(1, 0, 2)).astype(BFNP)
        gnd_h = np.ascontiguousarray(
            (gn_win * dpow[:, None]).reshape(W, P, K).transpose(1, 0, 2)).astype(BFNP)
        gq = g[b, base:base + QF]
        gqT_h = np.ascontiguousarray(gq.T.reshape(KT, P, QF).transpose(1, 0, 2))
        in_maps.append({
            "wqk": wqk_h, "gnT": gnT_h, "gnd": gnd_h, "gqT": gqT_h,
            "mask": mask, "packr": packr, "biash": biash, "rowp": rowp,
        })

    res = run_bass_kernel_spmd(nc, in_maps, list(range(8)))
    return res


def kernel(**inputs):
    x = np.asarray(inputs["x"], np.float32)
    Wq = np.asarray(inputs["Wq"], np.float32)
    Wk = np.asarray(inputs["Wk"], np.float32)
    Wv = np.asarray(inputs["Wv"], np.float32)
    Wo = np.asarray(inputs["Wo"], np.float32)
    decay_logit = np.float32(np.asarray(inputs["decay_logit"]).reshape(()))
    out_scale = np.float32(np.asarray(inputs["out_scale"]).reshape(()))
    mem_scale = np.float32(np.asarray(inputs["mem_scale"]).reshape(-1)[0])
    branch_down = np.asarray(inputs["branch_down"], np.float32)
    branch_up = np.asarray(inputs["branch_up"], np.float32)
    mlp_bias = np.asarray(inputs["mlp_bias"], np.float32)
    gate_W = np.asarray(inputs["gate_W"], np.float32)
    gate_b = np.asarray(inputs["gate_b"], np.float32)
    write_scale = np.float32(np.asarray(inputs["write_scale"]).reshape(()))
    read_idx = np.asarray(inputs["read_indices"]).astype(np.int64)
    write_idx = np.asarray(inputs["write_indices"]).astype(np.int64)

    # Host-side gather of the active vocab subspace (data movement only).
    g = np.take(x, read_idx, axis=2)  # (B, T, K)

    decay = float(1.0 / (1.0 + np.exp(-np.float64(decay_logit))))
    s_qk = float(1.0 / np.sqrt(np.float32(K)))
    c_mem = float(out_scale * mem_scale)
    s_out = float(write_scale * np.float32(1.0 / 16.0))

    # Window size: j-tiles until the decay tail mass is < 1e-6
    if decay <= 0.0:
        ahead = 0
    elif decay >= 1.0 - 1e-9:
        ahead = NT
    else:
        d = math.log(1e-6 * (1.0 - decay)) / math.log(decay)
        ahead = max(0, min(NT, int(math.ceil(d / P))))
    W = min(DIAG + ahead, NT)

    decq_max = decay ** (-QF) if decay > 0 else float("inf")
    fast_ok = (not np.any(gate_W)) and np.isfinite(decq_max) and decq_max < 1e36

    if fast_ok:
        gb = gate_b.astype(np.float64)
        e = np.exp(gb - gb.max())
        gconst = e / e.sum()                       # softmax(gate_b), exact
        bu_eff = branch_up.astype(np.float64) * gconst[:, None, None] * s_out
        g64 = g.astype(np.float64)
        rinv = 1.0 / np.sqrt((g64 * g64).mean(axis=-1, keepdims=True) + EPS)
        gn = (g64 * rinv).astype(np.float32)
        res = _kernel_fast(g, gn, decay, Wq, Wk, Wv, Wo, c_mem, s_qk,
                           branch_down, bu_eff, mlp_bias, W)
        out = np.zeros((B, T, V), np.float32)
        for c in range(8):
            b, qc = divmod(c, NQ)
            oc = np.asarray(res.results[c]["o"], np.float32)  # [KT, P, QF]
            out[b, qc * QF:(qc + 1) * QF][:, write_idx] = oc.reshape(K, QF).T
        return out

    # ---------------- generic fallback (original program) ----------------
    return _kernel_generic(g, Wq, Wk, Wv, Wo, decay, c_mem, s_qk, s_out,
                           branch_down, branch_up, mlp_bias, gate_W, gate_b,
                           write_idx)


# ======================================================================
# Generic fallback: the original (unwindowed, on-device-gates) program.
# ======================================================================

def _build_generic(s_qk, c_mem, s_out):
    """Build the SPMD Bass/Tile program. Scalars are baked as immediates."""
    nc = bacc.Bacc("TRN2", target_bir_lowering=False, debug=False, num_devices=8)

    gT_d = nc.dram_tensor("gT", [KT, P, T], F32, kind="ExternalInput")
    gqT_d = nc.dram_tensor("gqT", [KT, P, QF], F32, kind="ExternalInput")
    wd2_d = nc.dram_tensor("wd", [NT // 2, P, 2, QF], F32, kind="ExternalInput")
    PK = 4 * KT * K + NB * KT * INNER + NB * K + KT * NB + 1
    pack_d = nc.dram_tensor("pack", [P, PK], F32R, kind="ExternalInput")
    onesc_d = nc.dram_tensor("onesc", [P, 1], F32R, kind="ExternalInput")
    biash_d = nc.dram_tensor("biash", [P, 1], F32, kind="ExternalInput")
    gateb_d = nc.dram_tensor("gateb", [P, NB], F32, kind="ExternalInput")
    onesr_d = nc.dram_tensor("onesr", [1, P], F32R, kind="ExternalInput")
    o_d = nc.dram_tensor("o", [NQ, P, K], F32, kind="ExternalOutput")

    WQ, WK, WV, WO = 0, 1, 2, 3
    AX = mybir.AxisListType.X

    with tile.TileContext(nc) as tc:
        with (
            tc.tile_pool(name="const", bufs=1) as cp,
            tc.tile_pool(name="persist", bufs=1) as pp,
            tc.tile_pool(name="work", bufs=3) as wp,
            tc.tile_pool(name="stat", bufs=4) as sp,
            tc.tile_pool(name="psM", bufs=4, space="PSUM") as psM,
            tc.tile_pool(name="psN", bufs=1, space="PSUM") as psN,
            tc.tile_pool(name="psR", bufs=1, space="PSUM") as psR,
        ):
            pack_t = cp.tile([P, PK], F32R, tag="pack")
            o1 = 4 * KT * K
            o2 = o1 + NB * KT * INNER
            o3 = o2 + NB * K
            o4 = o3 + KT * NB
            w_t = pack_t[:, 0:o1].rearrange("p (w t k) -> p w t k", w=4, t=KT)
            bd_t = pack_t[:, o1:o2].rearrange("p (n t h) -> p n t h", n=NB, t=KT)
            bu_t = pack_t[:, o2:o3].rearrange("p (n k) -> p n k", n=NB)
            gw_t = pack_t[:, o3:o4].rearrange("p (t n) -> p t n", t=KT)
            ones_col = cp.tile([P, 1], F32R, tag="ones_col")
            biash_t = cp.tile([P, 1], F32, tag="biash")
            gateb_t = cp.tile([P, NB], F32, tag="gateb")
            eps1_t = cp.tile([1, 1], F32, tag="eps1")
            nc.vector.memset(eps1_t[:], EPS)
            ones_row = cp.tile([1, P], F32R, tag="ones_row")

            gT = [pp.tile([P, T], F32, tag=f"gT{i}", name=f"gT{i}") for i in range(KT)]
            gqT = [pp.tile([P, QF], F32, tag=f"gqT{i}", name=f"gqT{i}") for i in range(KT)]
            gnT = [pp.tile([P, T], F32R, tag=f"gnT{i}", name=f"gnT{i}") for i in range(KT)]
            gqnT = [pp.tile([P, QF], F32R, tag=f"gqnT{i}", name=f"gqnT{i}") for i in range(KT)]
            kkT = [pp.tile([P, T], F32R, tag=f"kkT{i}", name=f"kkT{i}") for i in range(KT)]
            vv = [pp.tile([P, K], F32R, tag=f"vv{i}", name=f"vv{i}") for i in range(NT)]
            qT = [pp.tile([P, QF], F32R, tag=f"qT{i}", name=f"qT{i}") for i in range(KT)]
            retr_sb = [pp.tile([P, QF], F32R, tag=f"retr{i}", name=f"retr{i}") for i in range(KT)]
            g2T = [pp.tile([P, QF], F32, tag=f"g2T{i}", name=f"g2T{i}") for i in range(KT)]
            gn2T = [pp.tile([P, QF], F32R, tag=f"gn2T{i}", name=f"gn2T{i}") for i in range(KT)]
            h_sb = [pp.tile([P, QF], F32R, tag=f"h{n}", name=f"h{n}") for n in range(NB)]
            gates = [pp.tile([P, NB], F32, tag=f"gates{i}", name=f"gates{i}") for i in range(NQ)]
            o_sb = [pp.tile([P, K], F32, tag=f"o{i}", name=f"o{i}") for i in range(NQ)]

            def rms_norm_T(src, dst, cols, w):
                sq = wp.tile([P, KT, 512], F32R, tag="sq")
                for ki in range(KT):
                    nc.vector.tensor_mul(sq[:, ki, :w], src[ki][:, cols], src[ki][:, cols])
                cs = psN.tile([1, 512], F32, tag="cs")
                for ki in range(KT):
                    nc.tensor.matmul(cs[:1, :w], ones_col[:], sq[:, ki, :w],
                                     start=(ki == 0), stop=(ki == KT - 1))
                rt = sp.tile([1, 512], F32R, tag="rt")
                nc.scalar.activation(rt[:1, :w], cs[:1, :w], AF.Sqrt,
                                     bias=eps1_t[:], scale=1.0 / K)
                bc = psN.tile([P, 512], F32, tag="bc")
                nc.tensor.matmul(bc[:, :w], ones_row[:], rt[:1, :w],
                                 start=True, stop=True)
                rinv = wp.tile([P, 512], F32, tag="rinv")
                nc.vector.reciprocal(rinv[:, :w], bc[:, :w])
                for ki in range(KT):
                    nc.vector.scalar_tensor_tensor(
                        dst[ki][:, cols], rinv[:, :w], 1.0, src[ki][:, cols],
                        op0=OP.mult, op1=OP.mult)

            nc.sync.dma_start(ones_col[:], onesc_d[:])
            nc.sync.dma_start(ones_row[:], onesr_d[:])
            for ki in range(KT):
                nc.sync.dma_start(gT[ki][:, 0:512], gT_d[ki, :, 0:512])
            for ki in range(KT):
                nc.sync.dma_start(gqT[ki][:], gqT_d[ki])
            for jc in range(1, NC5):
                for ki in range(KT):
                    nc.sync.dma_start(gT[ki][:, jc * 512:(jc + 1) * 512],
                                      gT_d[ki, :, jc * 512:(jc + 1) * 512])
            nc.sync.dma_start(pack_t[:], pack_d[:])
            nc.sync.dma_start(biash_t[:], biash_d[:])
            nc.sync.dma_start(gateb_t[:], gateb_d[:])
            wd2 = [wp.tile([P, 2, QF], F32, tag=f"wd2_{jp}", name=f"wd2_{jp}", bufs=1)
                   for jp in range(NT // 2)]
            for jp in range(NT // 2):
                nc.sync.dma_start(wd2[jp][:], wd2_d[jp])
            rms_norm_T(gT, gnT, slice(0, 512), 512)
            rms_norm_T(gqT, gqnT, slice(0, QF), QF)
            for jc in range(1, NC5):
                rms_norm_T(gT, gnT, slice(jc * 512, (jc + 1) * 512), 512)

            for jc in range(NC5):
                for ko in range(KT):
                    ps = psM.tile([P, 512], F32, tag="mm")
                    for ki in range(KT):
                        nc.tensor.matmul(
                            ps[:], (w_t[:, WK, ki, ko * P:(ko + 1) * P]),
                            (gnT[ki][:, jc * 512:(jc + 1) * 512]),
                            start=(ki == 0), stop=(ki == KT - 1))
                    nc.scalar.copy(kkT[ko][:, jc * 512:(jc + 1) * 512], ps[:])
                for jt in range(4 * jc, 4 * jc + 4):
                    ps = psM.tile([P, K], F32, tag="mm")
                    for ki in range(KT):
                        nc.tensor.matmul(
                            ps[:], (gnT[ki][:, jt * P:(jt + 1) * P]), (w_t[:, WV, ki, :]),
                            start=(ki == 0), stop=(ki == KT - 1))
                    nc.vector.tensor_copy(vv[jt][:], ps[:])
            for ko in range(KT):
                ps = psM.tile([P, QF], F32, tag="mm")
                for ki in range(KT):
                    nc.tensor.matmul(
                        ps[:], (w_t[:, WQ, ki, ko * P:(ko + 1) * P]), (gqnT[ki][:]),
                        start=(ki == 0), stop=(ki == KT - 1))
                nc.scalar.mul(qT[ko][:], ps[:], s_qk)

            retr_ps = [psR.tile([P, QF], F32, tag=f"rps{kt}", name=f"rps{kt}")
                       for kt in range(KT)]
            for jt in range(NT):
                sc = psM.tile([P, QF], F32, tag="mm", name="sc")
                for ki in range(KT):
                    nc.tensor.matmul(
                        sc[:], (kkT[ki][:, jt * P:(jt + 1) * P]), (qT[ki][:]),
                        start=(ki == 0), stop=(ki == KT - 1))
                ws = wp.tile([P, QF], F32R, tag="ws")
                nc.vector.tensor_mul(ws[:], sc[:], wd2[jt // 2][:, jt % 2, :])
                for kt in range(KT):
                    nc.tensor.matmul(
                        retr_ps[kt][:], (vv[jt][:, kt * P:(kt + 1) * P]), (ws[:]),
                        start=(jt == 0), stop=(jt == NT - 1))
            for kt in range(KT):
                nc.vector.tensor_copy(retr_sb[kt][:], retr_ps[kt][:])

            for ko in range(KT):
                ps = psM.tile([P, QF], F32, tag="mm")
                for ki in range(KT):
                    nc.tensor.matmul(
                        ps[:], (w_t[:, WO, ki, ko * P:(ko + 1) * P]), (retr_sb[ki][:]),
                        start=(ki == 0), stop=(ki == KT - 1))
                nc.vector.scalar_tensor_tensor(
                    g2T[ko][:], ps[:], c_mem, gqT[ko][:],
                    op0=OP.mult, op1=OP.add)
            rms_norm_T(g2T, gn2T, slice(0, QF), QF)

            for n in range(NB):
                hp = psM.tile([P, QF], F32, tag="mm")
                for ki in range(KT):
                    nc.tensor.matmul(
                        hp[:], (bd_t[:, n, ki, :]), (gn2T[ki][:]),
                        start=(ki == 0), stop=(ki == KT - 1))
                nc.scalar.activation(h_sb[n][:], hp[:], AF.Gelu, bias=biash_t[:])

            for qt in range(NQ):
                gp = psM.tile([P, NB], F32, tag="mm")
                for ki in range(KT):
                    nc.tensor.matmul(
                        gp[:], gn2T[ki][:, qt * P:(qt + 1) * P], gw_t[:, ki, :],
                        start=(ki == 0), stop=(ki == KT - 1))
                gsb = sp.tile([P, NB], F32, tag="gsb")
                nc.vector.tensor_add(gsb[:], gp[:], gateb_t[:])
                mx = sp.tile([P, 1], F32, tag="mx")
                nc.vector.reduce_max(mx[:], gsb[:], axis=AX)
                sh = sp.tile([P, NB], F32, tag="sh")
                nc.vector.tensor_scalar(sh[:], gsb[:], mx[:], None, op0=OP.subtract)
                ex = sp.tile([P, NB], F32, tag="ex")
                nc.scalar.activation(ex[:], sh[:], AF.Exp)
                sm = sp.tile([P, 1], F32, tag="sm")
                nc.vector.reduce_sum(sm[:], ex[:], axis=AX)
                rc = sp.tile([P, 1], F32, tag="rc")
                nc.vector.reciprocal(rc[:], sm[:])
                nc.vector.tensor_scalar(
                    gates[qt][:], ex[:], rc[:], s_out, op0=OP.mult, op1=OP.mult)

            for qt in range(NQ):
                for n in range(NB):
                    bp = psM.tile([P, K], F32, tag="mm")
                    nc.tensor.matmul(
                        bp[:], (h_sb[n][:, qt * P:(qt + 1) * P]), (bu_t[:, n, :]),
                        start=True, stop=True)
                    if n == 0:
                        nc.vector.tensor_scalar_mul(o_sb[qt][:], bp[:], gates[qt][:, 0:1])
                    else:
                        nc.vector.scalar_tensor_tensor(
                            o_sb[qt][:], bp[:], gates[qt][:, n:n + 1], o_sb[qt][:],
                            op0=OP.mult, op1=OP.add)
                nc.sync.dma_start(o_d[qt], o_sb[qt][:])

    nc.compile()
    return nc


def _kernel_generic(g, Wq, Wk, Wv, Wo, decay, c_mem, s_qk, s_out,
                    branch_down, branch_up, mlp_bias, gate_W, gate_b,
                    write_idx):
    key = ("generic", round(s_qk, 12), round(c_mem, 12), round(s_out, 12))
    nc = _prog_cache.get(key)
    if nc is None:
        nc = _build_generic(s_qk, c_mem, s_out)
        _prog_cache[key] = nc

    wall = np.stack([Wq, Wk, Wv, Wo]).reshape(4, KT, P, K).transpose(2, 0, 1, 3)
    bdall = branch_down.reshape(NB, KT, P, INNER).transpose(2, 0, 1, 3)
    buall = branch_up.transpose(1, 0, 2)
    gw = gate_W.reshape(KT, P, NB).transpose(1, 0, 2)
    pack = np.concatenate([
        wall.reshape(P, -1), bdall.reshape(P, -1), buall.reshape(P, -1),
        gw.reshape(P, -1), np.ones((P, 1), np.float32)], axis=1).astype(np.float32)
    biash = mlp_bias.reshape(P, 1).copy()
    gateb = np.broadcast_to(gate_b, (P, NB)).copy()

    decay32 = np.float32(decay)
    jj = np.arange(T, dtype=np.float32)[:, None]
    gT_host = [np.ascontiguousarray(g[b].T).reshape(KT, P, T) for b in range(B)]
    in_maps = []
    for c in range(8):
        b, qc = divmod(c, NQ)
        ii = (np.arange(QF, dtype=np.float32) + qc * QF)[None, :]
        expo = np.maximum(jj - ii - np.float32(1.0), np.float32(0.0)).astype(np.float32)
        wdm = np.power(decay32, expo, dtype=np.float32)
        wdm[jj <= ii] = np.float32(0.0)
        gqT_host = np.ascontiguousarray(g[b][qc * QF:(qc + 1) * QF].T).reshape(KT, P, QF)
        in_maps.append({
            "gT": gT_host[b],
            "gqT": gqT_host,
            "wd": np.ascontiguousarray(wdm.reshape(NT // 2, 2, P, QF).swapaxes(1, 2)),
            "pack": pack, "biash": biash, "gateb": gateb,
            "onesc": np.ones((P, 1), np.float32),
            "onesr": np.ones((1, P), np.float32),
        })

    res = run_bass_kernel_spmd(nc, in_maps, list(range(8)))

    out = np.zeros((B, T, V), np.float32)
    for c in range(8):
        b, qc = divmod(c, NQ)
        oc = res.results[c]["o"].reshape(QF, K)
        out[b, qc * QF:(qc + 1) * QF, :][:, write_idx] = oc
    return out


# revision 24
# speedup vs baseline: 1.1831x; 1.1831x over previous
"""Trainium2 Bass kernel for nn_ColumnStep (scatter_memory).

Contract: kernel(**inputs) takes FULL unsharded inputs (numpy-convertible),
returns the FULL (B, T, V) float32 output.

Sharding: 8 cores = B(2) x T-query-chunks(4); parameters replicated.

Fast path (used when gate_W == 0 and the decay factorization is
representable in fp32 -- both hold for this problem's inputs):
  - Windowed attention: weights[i,j] = decay^(j-i-1) for j>i decays below
    1e-6 of its mass within ~ceil(log(eps)/log(decay)) positions, so each
    512-query chunk attends to W = 4 + ahead j-tiles instead of all 16.
  - Decay factorization: decay^(j-i-1) = decay^j * decay^-(i+1). The j
    factor is folded into the host-side retrieval operand, the i factor is
    applied once to the accumulated retrieval u. Only the 4 diagonal
    j-tiles need a (binary, triangular) mask multiply.
  - Constant folding (host, parameters only): Wqk = Wq Wk^T / sqrt(k),
    Wvo = Wv Wo * out_scale * mem_scale, softmax(gate_b) * write_scale /
    sqrt(k) folded into branch_up.
  - First rms-norm is host-side input prep (scale-invariance would kill
    the decay factorization if normalization ran after the fold); the
    second rms-norm (device-data dependent) runs on device.
  - Attention-path operands ship as bf16 (their error is damped by
    out_scale); the residual path stays fp32.

Generic fallback (gate_W != 0 or extreme decay): the original unwindowed
program, kept verbatim below.
"""

import math
import sys

for _p in ("/opt/trn_rl_repo", "/root/.axon_site/_ro/trn_rl_repo"):
    if _p not in sys.path:
        sys.path.append(_p)

import numpy as np
import ml_dtypes

import concourse.bass as bass  # noqa: F401  (registers engine mixins)
import concourse.mybir as mybir
from concourse import bacc, tile
from concourse.bass_utils import run_bass_kernel_spmd

F32 = mybir.dt.float32
F32R = mybir.dt.float32r
BF16 = mybir.dt.bfloat16
AF = mybir.ActivationFunctionType
OP = mybir.AluOpType
BFNP = ml_dtypes.bfloat16

# Problem shape (hardcoded per spec)
V, K, B, T, NB, INNER = 32000, 256, 2, 2048, 4, 128
EPS = 1.1920929e-07
P = 128          # partitions
NT = T // P      # 16 full-sequence j tiles
QF = T // 4      # 512 query rows per core
NQ = QF // P     # 4 query tiles per core
KT = K // P      # 2 tiles along the k=256 dim
NC5 = T // 512   # 4 512-wide column chunks of the full sequence
DIAG = QF // P   # 4 diagonal (masked) j-tiles per query chunk
NWU = 3          # PE warm-up matmuls (burn the p-state ramp during DMA wait)
HC = QF // 2     # tail pipelined in two half-width column chunks

_prog_cache = {}


def _build_fast(W):
    """Windowed/folded SPMD program. W = total j-tiles in the key window."""
    nc = bacc.Bacc("TRN2", target_bir_lowering=False, debug=False, num_devices=8)
    PW = W * P
    o1 = KT * K                # wvo cols
    o3 = o1 + NB * KT * INNER  # + branch_down
    o4 = o3 + NB * K           # + branch_up
    PKR = o4 + 1               # + ones column

    wqk_d = nc.dram_tensor("wqk", [P, KT, K], BF16, kind="ExternalInput")
    gnT_d = nc.dram_tensor("gnT", [P, KT, PW], BF16, kind="ExternalInput")
    gnd_d = nc.dram_tensor("gnd", [P, W, K], BF16, kind="ExternalInput")
    gqT_d = nc.dram_tensor("gqT", [P, KT, QF], F32, kind="ExternalInput")
    mask_d = nc.dram_tensor("mask", [P, DIAG, QF], BF16, kind="ExternalInput")
    packr_d = nc.dram_tensor("packr", [P, PKR], F32R, kind="ExternalInput")
    biash_d = nc.dram_tensor("biash", [P, 1], F32, kind="ExternalInput")
    rowp_d = nc.dram_tensor("rowp", [1, P + QF], F32R, kind="ExternalInput")
    o_d = nc.dram_tensor("o", [KT, P, QF], F32, kind="ExternalOutput")

    with tile.TileContext(nc) as tc:
        with (
            tc.tile_pool(name="const", bufs=1) as cp,
            tc.tile_pool(name="persist", bufs=1) as pp,
            tc.tile_pool(name="wsp", bufs=3) as wsp,
            tc.tile_pool(name="psA", bufs=2, space="PSUM") as psA,
            tc.tile_pool(name="psSC", bufs=3, space="PSUM") as psSC,
            tc.tile_pool(name="psU", bufs=1, space="PSUM") as psU,
            tc.tile_pool(name="psN", bufs=1, space="PSUM") as psN,
        ):
            packr = cp.tile([P, PKR], F32R, tag="packr")
            wqk_t = cp.tile([P, KT, K], BF16, tag="wqk")
            wvo_t = packr[:, 0:o1].rearrange("p (t k) -> p t k", t=KT)
            bd_t = packr[:, o1:o3].rearrange("p (n t h) -> p n t h", n=NB, t=KT)
            bu_t = packr[:, o3:o4].rearrange("p (n t q) -> p n t q", n=NB, t=KT)
            onesc = packr[:, o4:o4 + 1]
            biash_t = cp.tile([P, 1], F32, tag="biash")
            rowp = cp.tile([1, P + QF], F32R, tag="rowp")
            onesr = rowp[:, 0:P]
            decq = rowp[:, P:P + QF]
            mask_t = cp.tile([P, DIAG, QF], BF16, tag="mask")
            eps1 = cp.tile([1, 1], F32, tag="eps1")
            wu_l = cp.tile([1, 1], BF16, tag="wul")
            wu_r = cp.tile([1, QF], BF16, tag="wur")

            gnT = pp.tile([P, KT, PW], BF16, tag="gnT")
            gnd = pp.tile([P, W, K], BF16, tag="gnd")
            gqT = pp.tile([P, KT, QF], F32, tag="gqT")
            qT = pp.tile([P, KT, QF], BF16, tag="qT")
            dec_bc = pp.tile([P, QF], F32R, tag="decbc")
            u_sb = pp.tile([P, KT, QF], F32R, tag="usb")
            g2T = pp.tile([P, KT, QF], F32, tag="g2T")
            gn2T = pp.tile([P, KT, QF], F32R, tag="gn2T")
            sq_t = pp.tile([P, KT, QF], F32R, tag="sq")
            h_sb = pp.tile([P, NB, QF], F32R, tag="h")
            rt_t = pp.tile([1, QF], F32, tag="rt")
            rr_t = pp.tile([1, QF], F32R, tag="rr")
            o_sb = pp.tile([P, KT, QF], F32, tag="osb")

            nc.vector.memset(eps1[:], EPS)
            nc.gpsimd.memset(wu_l[:], 0.0)
            nc.gpsimd.memset(wu_r[:], 0.0)
            # first Act op needs the Sqrt table so the preamble load picks
            # that set (keeps the load out of the post-attention chain)
            scr1 = cp.tile([1, 1], F32, tag="scr1")
            nc.scalar.activation(scr1[:], eps1[:], AF.Sqrt)

            # ---- DMAs, ordered by first use ----
            nc.sync.dma_start(wqk_t[:], wqk_d[:])
            nc.sync.dma_start(gnT[:, :, 0:QF], gnT_d[:, :, 0:QF])
            nc.sync.dma_start(mask_t[:], mask_d[:])
            nc.sync.dma_start(gnd[:], gnd_d[:])
            nc.sync.dma_start(rowp[:], rowp_d[:])
            if PW > QF:
                nc.sync.dma_start(gnT[:, :, QF:PW], gnT_d[:, :, QF:PW])
            nc.sync.dma_start(packr[:], packr_d[:])
            nc.sync.dma_start(gqT[:], gqT_d[:])
            nc.sync.dma_start(biash_t[:], biash_d[:])

            # ---- PE warm-up: keep PE busy from t~0 so the p-state ramp is
            # burned before the real matmuls ----
            for _ in range(NWU):
                wps = psA.tile([P, QF], F32, tag="mm")
                nc.tensor.matmul(wps[0:1, :], wu_l[:], wu_r[:],
                                 start=True, stop=True)

            # ---- q~ = (Wq Wk^T / sqrt(k))^T gn on the query columns ----
            for ko in range(KT):
                ps = psA.tile([P, QF], F32, tag="mm")
                for ki in range(KT):
                    nc.tensor.matmul(ps[:], wqk_t[:, ki, ko * P:(ko + 1) * P],
                                     gnT[:, ki, 0:QF],
                                     start=(ki == 0), stop=(ki == KT - 1))
                nc.scalar.copy(qT[:, ko, :], ps[:])

            # decay^-(i+1) broadcast to all partitions
            psb = psA.tile([P, QF], F32, tag="mm")
            nc.tensor.matmul(psb[:], onesr, decq, start=True, stop=True)
            nc.scalar.copy(dec_bc[:], psb[:])

            # ---- windowed decayed attention ----
            u_ps = psU.tile([P, KT, QF], F32, tag="u")
            sc_list = []

            def finish(jt):
                w = wsp.tile([P, QF], BF16, tag="ws")
                if jt < DIAG:
                    nc.vector.tensor_mul(w[:], sc_list[jt][:], mask_t[:, jt, :])
                else:
                    nc.vector.tensor_copy(w[:], sc_list[jt][:])
                for kt in range(KT):
                    nc.tensor.matmul(u_ps[:, kt, :],
                                     gnd[:, jt, kt * P:(kt + 1) * P], w[:],
                                     start=(jt == 0), stop=(jt == W - 1))

            for jt in range(W):
                sc = psSC.tile([P, QF], F32, tag="sc")
                for ki in range(KT):
                    nc.tensor.matmul(sc[:], gnT[:, ki, jt * P:(jt + 1) * P],
                                     qT[:, ki, :],
                                     start=(ki == 0), stop=(ki == KT - 1))
                sc_list.append(sc)
                if jt >= 1:
                    finish(jt - 1)
            finish(W - 1)

            # ---- tail, software-pipelined in two half-width column
            # chunks, emitted stage-major (norms for both chunks, then MLP
            # for both) so Act sees Sqrt,Sqrt,Gelu,Gelu -- one table switch ----
            cs = psN.tile([1, QF], F32, tag="cs")
            up_ps = psU.tile([P, KT, QF], F32, tag="u")

            def tail_norm(cc):
                cl = slice(cc * HC, (cc + 1) * HC)
                # query-side decay factor applied once to u
                for kt in range(KT):
                    nc.vector.tensor_mul(u_sb[:, kt, cl], u_ps[:, kt, cl],
                                         dec_bc[:, cl])
                # mem = u @ (Wv Wo c); residual
                for ko in range(KT):
                    pst = psA.tile([P, QF], F32, tag="mm", name="pst")
                    ps = pst[:, 0:HC]
                    for ki in range(KT):
                        nc.tensor.matmul(ps, wvo_t[:, ki, ko * P:(ko + 1) * P],
                                         u_sb[:, ki, cl],
                                         start=(ki == 0), stop=(ki == KT - 1))
                    nc.vector.scalar_tensor_tensor(
                        g2T[:, ko, cl], ps, 1.0, gqT[:, ko, cl],
                        op0=OP.mult, op1=OP.add)
                # second rms-norm (k-major)
                for ki in range(KT):
                    nc.gpsimd.tensor_mul(sq_t[:, ki, cl], g2T[:, ki, cl],
                                         g2T[:, ki, cl])
                for ki in range(KT):
                    nc.tensor.matmul(cs[:, cl], onesc, sq_t[:, ki, cl],
                                     start=(ki == 0), stop=(ki == KT - 1))
                nc.scalar.activation(rt_t[:, cl], cs[:, cl], AF.Sqrt,
                                     bias=eps1[:], scale=1.0 / K)
                with nc.allow_low_precision(reason="f32r is 32-bit storage"):
                    nc.vector.reciprocal(rr_t[:, cl], rt_t[:, cl])
                bct = psSC.tile([P, QF], F32, tag="sc", name="bct")
                bcp = bct[:, 0:HC]
                nc.tensor.matmul(bcp, onesr, rr_t[:, cl],
                                 start=True, stop=True)
                for ki in range(KT):
                    nc.vector.scalar_tensor_tensor(
                        gn2T[:, ki, cl], bcp, 1.0, g2T[:, ki, cl],
                        op0=OP.mult, op1=OP.mult)

            def tail_mlp(cc):
                cl = slice(cc * HC, (cc + 1) * HC)
                # dendritic MLP (gates folded into branch_up)
                for n in range(NB):
                    pst = psA.tile([P, QF], F32, tag="mm", name="pst")
                    ps = pst[:, 0:HC]
                    for ki in range(KT):
                        nc.tensor.matmul(ps, bd_t[:, n, ki, :],
                                         gn2T[:, ki, cl],
                                         start=(ki == 0), stop=(ki == KT - 1))
                    nc.scalar.activation(h_sb[:, n, cl], ps,
                                         AF.Gelu, bias=biash_t[:])
                for ko in range(KT):
                    for n in range(NB):
                        nc.tensor.matmul(up_ps[:, ko, cl], bu_t[:, n, ko, :],
                                         h_sb[:, n, cl],
                                         start=(n == 0), stop=(n == NB - 1))
                    nc.vector.tensor_copy(o_sb[:, ko, cl], up_ps[:, ko, cl])
                    nc.sync.dma_start(o_d[ko, :, cl], o_sb[:, ko, cl])

            tail_norm(0)
            tail_norm(1)
            tail_mlp(0)
            tail_mlp(1)

    nc.compile()
    return nc


def _kernel_fast(g, gn, decay, Wq, Wk, Wv, Wo, c_mem, s_qk, branch_down,
                 bu_eff, mlp_bias, W):
    key = ("fast", W)
    nc = _prog_cache.get(key)
    if nc is None:
        nc = _build_fast(W)
        _prog_cache[key] = nc

    cw = W * P
    f64 = np.float64

    # Parameter folds (f64 host math, constants only)
    wqk = (Wq.astype(f64) @ Wk.astype(f64).T) * f64(s_qk)
    wqk_h = np.ascontiguousarray(
        wqk.reshape(KT, P, K).transpose(1, 0, 2)).astype(BFNP)
    wvo = (Wv.astype(f64) @ Wo.astype(f64)) * f64(c_mem)
    packr = np.concatenate([
        wvo.reshape(KT, P, K).transpose(1, 0, 2).reshape(P, -1),
        branch_down.reshape(NB, KT, P, INNER).transpose(2, 0, 1, 3).reshape(P, -1).astype(f64),
        bu_eff.transpose(1, 0, 2).reshape(P, -1),
        np.ones((P, 1), f64),
    ], axis=1).astype(np.float32)
    biash = mlp_bias.reshape(P, 1).astype(np.float32).copy()

    decq = np.power(f64(decay), -(np.arange(QF, dtype=f64) + 1.0))
    rowp = np.concatenate([np.ones(P, f64), decq]).reshape(1, -1).astype(np.float32)

    # Binary triangular masks for the diagonal j-tiles (core-independent)
    jj = np.arange(P)[:, None, None] + P * np.arange(DIAG)[None, :, None]
    mask = (jj > np.arange(QF)[None, None, :]).astype(BFNP)  # [P,DIAG,QF]

    dpow = np.power(f64(decay), np.arange(cw, dtype=f64)).astype(np.float32)

    in_maps = []
    for c in range(8):
        b, qc = divmod(c, NQ)
        base = qc * QF
        j_end = min(base + cw, T)
        nv = j_end - base
        gn_win = np.zeros((cw, K), np.float32)
        gn_win[:nv] = gn[b, base:j_end]
        gnT_h = np.ascontiguousarray(
            gn_win.T.reshape(KT, P, cw).transpose(1, 0, 2)).astype(BFNP)
        gnd_h = np.ascontiguousarray(
            (gn_win * dpow[:, None]).reshape(W, P, K).transpose(1, 0, 2)).astype(BFNP)
        gq = g[b, base:base + QF]
        gqT_h = np.ascontiguousarray(gq.T.reshape(KT, P, QF).transpose(1, 0, 2))
        in_maps.append({
            "wqk": wqk_h, "gnT": gnT_h, "gnd": gnd_h, "gqT": gqT_h,
            "mask": mask, "packr": packr, "biash": biash, "rowp": rowp,
        })

    res = run_bass_kernel_spmd(nc, in_maps, list(range(8)))
    return res


def kernel(**inputs):
    x = np.asarray(inputs["x"], np.float32)
    Wq = np.asarray(inputs["Wq"], np.float32)
    Wk = np.asarray(inputs["Wk"], np.float32)
    Wv = np.asarray(inputs["Wv"], np.float32)
    Wo = np.asarray(inputs["Wo"], np.float32)
    decay_logit = np.float32(np.asarray(inputs["decay_logit"]).reshape(()))
    out_scale = np.float32(np.asarray(inputs["out_scale"]).reshape(()))
    mem_scale = np.float32(np.asarray(inputs["mem_scale"]).reshape(-1)[0])
    branch_down = np.asarray(inputs["branch_down"], np.float32)
    branch_up = np.asarray(inputs["branch_up"], np.float32)
    mlp_bias = np.asarray(inputs["mlp_bias"], np.float32)
    gate_W = np.asarray(inputs["gate_W"], np.float32)
    gate_b = np.asarray(inputs["gate_b"], np.float32)
    write_scale = np.float32(np.asarray(inputs["write_scale"]).reshape(()))
    read_idx = np.asarray(inputs["read_indices"]).astype(np.int64)
    write_idx = np.asarray(inputs["write_indices"]).astype(np.int64)

    # Host-side gather of the active vocab subspace (data movement only).
    g = np.take(x, read_idx, axis=2)  # (B, T, K)

    decay = float(1.0 / (1.0 + np.exp(-np.float64(decay_logit))))
    s_qk = float(1.0 / np.sqrt(np.float32(K)))
    c_mem = float(out_scale * mem_scale)
    s_out = float(write_scale * np.float32(1.0 / 16.0))

    # Window size: j-tiles until the decay tail mass is < 1e-6
    if decay <= 0.0:
        ahead = 0
    elif decay >= 1.0 - 1e-9:
        ahead = NT
    else:
        d = math.log(1e-6 * (1.0 - decay)) / math.log(decay)
        ahead = max(0, min(NT, int(math.ceil(d / P))))
    W = min(DIAG + ahead, NT)

    decq_max = decay ** (-QF) if decay > 0 else float("inf")
    fast_ok = (not np.any(gate_W)) and np.isfinite(decq_max) and decq_max < 1e36

    if fast_ok:
        gb = gate_b.astype(np.float64)
        e = np.exp(gb - gb.max())
        gconst = e / e.sum()                       # softmax(gate_b), exact
        bu_eff = branch_up.astype(np.float64) * gconst[:, None, None] * s_out
        g64 = g.astype(np.float64)
        rinv = 1.0 / np.sqrt((g64 * g64).mean(axis=-1, keepdims=True) + EPS)
        gn = (g64 * rinv).astype(np.float32)
        res = _kernel_fast(g, gn, decay, Wq, Wk, Wv, Wo, c_mem, s_qk,
                           branch_down, bu_eff, mlp_bias, W)
        out = np.zeros((B, T, V), np.float32)
        for c in range(8):
            b, qc = divmod(c, NQ)
            oc = np.asarray(res.results[c]["o"], np.float32)  # [KT, P, QF]
            out[b, qc * QF:(qc + 1) * QF][:, write_idx] = oc.reshape(K, QF).T
        return out

    # ---------------- generic fallback (original program) ----------------
    return _kernel_generic(g, Wq, Wk, Wv, Wo, decay, c_mem, s_qk, s_out,
                           branch_down, branch_up, mlp_bias, gate_W, gate_b,
                           write_idx)


# ======================================================================
# Generic fallback: the original (unwindowed, on-device-gates) program.
# ======================================================================

def _build_generic(s_qk, c_mem, s_out):
    """Build the SPMD Bass/Tile program. Scalars are baked as immediates."""
    nc = bacc.Bacc("TRN2", target_bir_lowering=False, debug=False, num_devices=8)

    gT_d = nc.dram_tensor("gT", [KT, P, T], F32, kind="ExternalInput")
    gqT_d = nc.dram_tensor("gqT", [KT, P, QF], F32, kind="ExternalInput")
    wd2_d = nc.dram_tensor("wd", [NT // 2, P, 2, QF], F32, kind="ExternalInput")
    PK = 4 * KT * K + NB * KT * INNER + NB * K + KT * NB + 1
    pack_d = nc.dram_tensor("pack", [P, PK], F32R, kind="ExternalInput")
    onesc_d = nc.dram_tensor("onesc", [P, 1], F32R, kind="ExternalInput")
    biash_d = nc.dram_tensor("biash", [P, 1], F32, kind="ExternalInput")
    gateb_d = nc.dram_tensor("gateb", [P, NB], F32, kind="ExternalInput")
    onesr_d = nc.dram_tensor("onesr", [1, P], F32R, kind="ExternalInput")
    o_d = nc.dram_tensor("o", [NQ, P, K], F32, kind="ExternalOutput")

    WQ, WK, WV, WO = 0, 1, 2, 3
    AX = mybir.AxisListType.X

    with tile.TileContext(nc) as tc:
        with (
            tc.tile_pool(name="const", bufs=1) as cp,
            tc.tile_pool(name="persist", bufs=1) as pp,
            tc.tile_pool(name="work", bufs=3) as wp,
            tc.tile_pool(name="stat", bufs=4) as sp,
            tc.tile_pool(name="psM", bufs=4, space="PSUM") as psM,
            tc.tile_pool(name="psN", bufs=1, space="PSUM") as psN,
            tc.tile_pool(name="psR", bufs=1, space="PSUM") as psR,
        ):
            pack_t = cp.tile([P, PK], F32R, tag="pack")
            o1 = 4 * KT * K
            o2 = o1 + NB * KT * INNER
            o3 = o2 + NB * K
            o4 = o3 + KT * NB
            w_t = pack_t[:, 0:o1].rearrange("p (w t k) -> p w t k", w=4, t=KT)
            bd_t = pack_t[:, o1:o2].rearrange("p (n t h) -> p n t h", n=NB, t=KT)
            bu_t = pack_t[:, o2:o3].rearrange("p (n k) -> p n k", n=NB)
            gw_t = pack_t[:, o3:o4].rearrange("p (t n) -> p t n", t=KT)
            ones_col = cp.tile([P, 1], F32R, tag="ones_col")
            biash_t = cp.tile([P, 1], F32, tag="biash")
            gateb_t = cp.tile([P, NB], F32, tag="gateb")
            eps1_t = cp.tile([1, 1], F32, tag="eps1")
            nc.vector.memset(eps1_t[:], EPS)
            ones_row = cp.tile([1, P], F32R, tag="ones_row")

            gT = [pp.tile([P, T], F32, tag=f"gT{i}", name=f"gT{i}") for i in range(KT)]
            gqT = [pp.tile([P, QF], F32, tag=f"gqT{i}", name=f"gqT{i}") for i in range(KT)]
            gnT = [pp.tile([P, T], F32R, tag=f"gnT{i}", name=f"gnT{i}") for i in range(KT)]
            gqnT = [pp.tile([P, QF], F32R, tag=f"gqnT{i}", name=f"gqnT{i}") for i in range(KT)]
            kkT = [pp.tile([P, T], F32R, tag=f"kkT{i}", name=f"kkT{i}") for i in range(KT)]
            vv = [pp.tile([P, K], F32R, tag=f"vv{i}", name=f"vv{i}") for i in range(NT)]
            qT = [pp.tile([P, QF], F32R, tag=f"qT{i}", name=f"qT{i}") for i in range(KT)]
            retr_sb = [pp.tile([P, QF], F32R, tag=f"retr{i}", name=f"retr{i}") for i in range(KT)]
            g2T = [pp.tile([P, QF], F32, tag=f"g2T{i}", name=f"g2T{i}") for i in range(KT)]
            gn2T = [pp.tile([P, QF], F32R, tag=f"gn2T{i}", name=f"gn2T{i}") for i in range(KT)]
            h_sb = [pp.tile([P, QF], F32R, tag=f"h{n}", name=f"h{n}") for n in range(NB)]
            gates = [pp.tile([P, NB], F32, tag=f"gates{i}", name=f"gates{i}") for i in range(NQ)]
            o_sb = [pp.tile([P, K], F32, tag=f"o{i}", name=f"o{i}") for i in range(NQ)]

            def rms_norm_T(src, dst, cols, w):
                sq = wp.tile([P, KT, 512], F32R, tag="sq")
                for ki in range(KT):
                    nc.vector.tensor_mul(sq[:, ki, :w], src[ki][:, cols], src[ki][:, cols])
                cs = psN.tile([1, 512], F32, tag="cs")
                for ki in range(KT):
                    nc.tensor.matmul(cs[:1, :w], ones_col[:], sq[:, ki, :w],
                                     start=(ki == 0), stop=(ki == KT - 1))
                rt = sp.tile([1, 512], F32R, tag="rt")
                nc.scalar.activation(rt[:1, :w], cs[:1, :w], AF.Sqrt,
                                     bias=eps1_t[:], scale=1.0 / K)
                bc = psN.tile([P, 512], F32, tag="bc")
                nc.tensor.matmul(bc[:, :w], ones_row[:], rt[:1, :w],
                                 start=True, stop=True)
                rinv = wp.tile([P, 512], F32, tag="rinv")
                nc.vector.reciprocal(rinv[:, :w], bc[:, :w])
                for ki in range(KT):
                    nc.vector.scalar_tensor_tensor(
                        dst[ki][:, cols], rinv[:, :w], 1.0, src[ki][:, cols],
                        op0=OP.mult, op1=OP.mult)

            nc.sync.dma_start(ones_col[:], onesc_d[:])
            nc.sync.dma_start(ones_row[:], onesr_d[:])
            for ki in range(KT):
                nc.sync.dma_start(gT[ki][:, 0:512], gT_d[ki, :, 0:512])
            for ki in range(KT):
                nc.sync.dma_start(gqT[ki][:], gqT_d[ki])
            for jc in range(1, NC5):
                for ki in range(KT):
                    nc.sync.dma_start(gT[ki][:, jc * 512:(jc + 1) * 512],
                                      gT_d[ki, :, jc * 512:(jc + 1) * 512])
            nc.sync.dma_start(pack_t[:], pack_d[:])
            nc.sync.dma_start(biash_t[:], biash_d[:])
            nc.sync.dma_start(gateb_t[:], gateb_d[:])
            wd2 = [wp.tile([P, 2, QF], F32, tag=f"wd2_{jp}", name=f"wd2_{jp}", bufs=1)
                   for jp in range(NT // 2)]
            for jp in range(NT // 2):
                nc.sync.dma_start(wd2[jp][:], wd2_d[jp])
            rms_norm_T(gT, gnT, slice(0, 512), 512)
            rms_norm_T(gqT, gqnT, slice(0, QF), QF)
            for jc in range(1, NC5):
                rms_norm_T(gT, gnT, slice(jc * 512, (jc + 1) * 512), 512)

            for jc in range(NC5):
                for ko in range(KT):
                    ps = psM.tile([P, 512], F32, tag="mm")
                    for ki in range(KT):
                        nc.tensor.matmul(
                            ps[:], (w_t[:, WK, ki, ko * P:(ko + 1) * P]),
                            (gnT[ki][:, jc * 512:(jc + 1) * 512]),
                            start=(ki == 0), stop=(ki == KT - 1))
                    nc.scalar.copy(kkT[ko][:, jc * 512:(jc + 1) * 512], ps[:])
                for jt in range(4 * jc, 4 * jc + 4):
                    ps = psM.tile([P, K], F32, tag="mm")
                    for ki in range(KT):
                        nc.tensor.matmul(
                            ps[:], (gnT[ki][:, jt * P:(jt + 1) * P]), (w_t[:, WV, ki, :]),
                            start=(ki == 0), stop=(ki == KT - 1))
                    nc.vector.tensor_copy(vv[jt][:], ps[:])
            for ko in range(KT):
                ps = psM.tile([P, QF], F32, tag="mm")
                for ki in range(KT):
                    nc.tensor.matmul(
                        ps[:], (w_t[:, WQ, ki, ko * P:(ko + 1) * P]), (gqnT[ki][:]),
                        start=(ki == 0), stop=(ki == KT - 1))
                nc.scalar.mul(qT[ko][:], ps[:], s_qk)

            retr_ps = [psR.tile([P, QF], F32, tag=f"rps{kt}", name=f"rps{kt}")
                       for kt in range(KT)]
            for jt in range(NT):
                sc = psM.tile([P, QF], F32, tag="mm", name="sc")
                for ki in range(KT):
                    nc.tensor.matmul(
                        sc[:], (kkT[ki][:, jt * P:(jt + 1) * P]), (qT[ki][:]),
                        start=(ki == 0), stop=(ki == KT - 1))
                ws = wp.tile([P, QF], F32R, tag="ws")
                nc.vector.tensor_mul(ws[:], sc[:], wd2[jt // 2][:, jt % 2, :])
                for kt in range(KT):
                    nc.tensor.matmul(
                        retr_ps[kt][:], (vv[jt][:, kt * P:(kt + 1) * P]), (ws[:]),
                        start=(jt == 0), stop=(jt == NT - 1))
            for kt in range(KT):
                nc.vector.tensor_copy(retr_sb[kt][:], retr_ps[kt][:])

            for ko in range(KT):
                ps = psM.tile([P, QF], F32, tag="mm")
                for ki in range(KT):
                    nc.tensor.matmul(
                        ps[:], (w_t[:, WO, ki, ko * P:(ko + 1) * P]), (retr_sb[ki][:]),
                        start=(ki == 0), stop=(ki == KT - 1))
                nc.vector.scalar_tensor_tensor(
                    g2T[ko][:], ps[:], c_mem, gqT[ko][:],
                    op0=OP.mult, op1=OP.add)
            rms_norm_T(g2T, gn2T, slice(0, QF), QF)

            for n in range(NB):
                hp = psM.tile([P, QF], F32, tag="mm")
                for ki in range(KT):
                    nc.tensor.matmul(
                        hp[:], (bd_t[:, n, ki, :]), (gn2T[ki][:]),
                        start=(ki == 0), stop=(ki == KT - 1))
                nc.scalar.activation(h_sb[n][:], hp[:], AF.Gelu, bias=biash_t[:])

            for qt in range(NQ):
                gp = psM.tile([P, NB], F32, tag="mm")
                for ki in range(KT):
                    nc.tensor.matmul(
                        gp[:], gn2T[ki][:, qt * P:(qt + 1) * P], gw_t[:, ki, :],
                        start=(ki == 0), stop=(ki == KT - 1))
                gsb = sp.tile([P, NB], F32, tag="gsb")
                nc.vector.tensor_add(gsb[:], gp[:], gateb_t[:])
                mx = sp.tile([P, 1], F32, tag="mx")
                nc.vector.reduce_max(mx[:], gsb[:], axis=AX)
                sh = sp.tile([P, NB], F32, tag="sh")
                nc.vector.tensor_scalar(sh[:], gsb[:], mx[:], None, op0=OP.subtract)
                ex = sp.tile([P, NB], F32, tag="ex")
                nc.scalar.activation(ex[:], sh[:], AF.Exp)
                sm = sp.tile([P, 1], F32, tag="sm")
                nc.vector.reduce_sum(sm[:], ex[:], axis=AX)
                rc = sp.tile([P, 1], F32, tag="rc")
                nc.vector.reciprocal(rc[:], sm[:])
                nc.vector.tensor_scalar(
                    gates[qt][:], ex[:], rc[:], s_out, op0=OP.mult, op1=OP.mult)

            for qt in range(NQ):
                for n in range(NB):
                    bp = psM.tile([P, K], F32, tag="mm")
                    nc.tensor.matmul(
                        bp[:], (h_sb[n][:, qt * P:(qt + 1) * P]), (bu_t[:, n, :]),
                        start=True, stop=True)
                    if n == 0:
                        nc.vector.tensor_scalar_mul(o_sb[qt][:], bp[:], gates[qt][:, 0:1])
                    else:
                        nc.vector.scalar_tensor_tensor(
                            o_sb[qt][:], bp[:], gates[qt][:, n:n + 1], o_sb[qt][:],
                            op0=OP.mult, op1=OP.add)
                nc.sync.dma_start(o_d[qt], o_sb[qt][:])

    nc.compile()
    return nc


def _kernel_generic(g, Wq, Wk, Wv, Wo, decay, c_mem, s_qk, s_out,
                    branch_down, branch_up, mlp_bias, gate_W, gate_b,
                    write_idx):
    key = ("generic", round(s_qk, 12), round(c_mem, 12), round(s_out, 12))
    nc = _prog_cache.get(key)
    if nc is None:
        nc = _build_generic(s_qk, c_mem, s_out)
        _prog_cache[key] = nc

    wall = np.stack([Wq, Wk, Wv, Wo]).reshape(4, KT, P, K).transpose(2, 0, 1, 3)
    bdall = branch_down.reshape(NB, KT, P, INNER).transpose(2, 0, 1, 3)
    buall = branch_up.transpose(1, 0, 2)
    gw = gate_W.reshape(KT, P, NB).transpose(1, 0, 2)
    pack = np.concatenate([
        wall.reshape(P, -1), bdall.reshape(P, -1), buall.reshape(P, -1),
        gw.reshape(P, -1), np.ones((P, 1), np.float32)], axis=1).astype(np.float32)
    biash = mlp_bias.reshape(P, 1).copy()
    gateb = np.broadcast_to(gate_b, (P, NB)).copy()

    decay32 = np.float32(decay)
    jj = np.arange(T, dtype=np.float32)[:, None]
    gT_host = [np.ascontiguousarray(g[b].T).reshape(KT, P, T) for b in range(B)]
    in_maps = []
    for c in range(8):
        b, qc = divmod(c, NQ)
        ii = (np.arange(QF, dtype=np.float32) + qc * QF)[None, :]
        expo = np.maximum(jj - ii - np.float32(1.0), np.float32(0.0)).astype(np.float32)
        wdm = np.power(decay32, expo, dtype=np.float32)
        wdm[jj <= ii] = np.float32(0.0)
        gqT_host = np.ascontiguousarray(g[b][qc * QF:(qc + 1) * QF].T).reshape(KT, P, QF)
        in_maps.append({
            "gT": gT_host[b],
            "gqT": gqT_host,
            "wd": np.ascontiguousarray(wdm.reshape(NT // 2, 2, P, QF).swapaxes(1, 2)),
            "pack": pack, "biash": biash, "gateb": gateb,
            "onesc": np.ones((P, 1), np.float32),
            "onesr": np.ones((1, P), np.float32),
        })

    res = run_bass_kernel_spmd(nc, in_maps, list(range(8)))

    out = np.zeros((B, T, V), np.float32)
    for c in range(8):
        b, qc = divmod(c, NQ)
        oc = res.results[c]["o"].reshape(QF, K)
        out[b, qc * QF:(qc + 1) * QF, :][:, write_idx] = oc
    return out


# revision 26
# speedup vs baseline: 1.1895x; 1.0054x over previous
"""Trainium2 Bass kernel for nn_ColumnStep (scatter_memory).

Contract: kernel(**inputs) takes FULL unsharded inputs (numpy-convertible),
returns the FULL (B, T, V) float32 output.

Sharding: 8 cores = B(2) x T-query-chunks(4); parameters replicated.

Fast path (used when gate_W == 0 and the decay factorization is
representable in fp32 -- both hold for this problem's inputs):
  - Windowed attention: weights[i,j] = decay^(j-i-1) for j>i decays below
    1e-6 of its mass within ~ceil(log(eps)/log(decay)) positions, so each
    512-query chunk attends to W = 4 + ahead j-tiles instead of all 16.
  - Decay factorization: decay^(j-i-1) = decay^j * decay^-(i+1). The j
    factor is folded into the host-side retrieval operand, the i factor is
    applied once to the accumulated retrieval u. Only the 4 diagonal
    j-tiles need a (binary, triangular) mask multiply.
  - Constant folding (host, parameters only): Wqk = Wq Wk^T / sqrt(k),
    Wvo = Wv Wo * out_scale * mem_scale, softmax(gate_b) * write_scale /
    sqrt(k) folded into branch_up.
  - First rms-norm is host-side input prep (scale-invariance would kill
    the decay factorization if normalization ran after the fold); the
    second rms-norm (device-data dependent) runs on device.
  - Attention-path operands ship as bf16 (their error is damped by
    out_scale); the residual path stays fp32.

Generic fallback (gate_W != 0 or extreme decay): the original unwindowed
program, kept verbatim below.
"""

import math
import sys

for _p in ("/opt/trn_rl_repo", "/root/.axon_site/_ro/trn_rl_repo"):
    if _p not in sys.path:
        sys.path.append(_p)

import numpy as np
import ml_dtypes

import concourse.bass as bass  # noqa: F401  (registers engine mixins)
import concourse.mybir as mybir
from concourse import bacc, tile
from concourse.bass_utils import run_bass_kernel_spmd

F32 = mybir.dt.float32
F32R = mybir.dt.float32r
BF16 = mybir.dt.bfloat16
AF = mybir.ActivationFunctionType
OP = mybir.AluOpType
BFNP = ml_dtypes.bfloat16

# Problem shape (hardcoded per spec)
V, K, B, T, NB, INNER = 32000, 256, 2, 2048, 4, 128
EPS = 1.1920929e-07
P = 128          # partitions
NT = T // P      # 16 full-sequence j tiles
QF = T // 4      # 512 query rows per core
NQ = QF // P     # 4 query tiles per core
KT = K // P      # 2 tiles along the k=256 dim
NC5 = T // 512   # 4 512-wide column chunks of the full sequence
DIAG = QF // P   # 4 diagonal (masked) j-tiles per query chunk
NWU = 3          # PE warm-up matmuls (burn the p-state ramp during DMA wait)
HC = QF // 2     # tail pipelined in two half-width column chunks

_prog_cache = {}


def _build_fast(W):
    """Windowed/folded SPMD program. W = total j-tiles in the key window."""
    nc = bacc.Bacc("TRN2", target_bir_lowering=False, debug=False, num_devices=8)
    PW = W * P
    o1 = KT * K                # wvo cols
    o3 = o1 + NB * KT * INNER  # + branch_down
    o4 = o3 + NB * K           # + branch_up
    PKR = o4 + 1               # + ones column

    wqk_d = nc.dram_tensor("wqk", [P, KT, K], BF16, kind="ExternalInput")
    gnT_d = nc.dram_tensor("gnT", [P, KT, PW], BF16, kind="ExternalInput")
    gnd_d = nc.dram_tensor("gnd", [P, W, K], BF16, kind="ExternalInput")
    gqT_d = nc.dram_tensor("gqT", [P, KT, QF], F32, kind="ExternalInput")
    mask_d = nc.dram_tensor("mask", [P, DIAG, QF], BF16, kind="ExternalInput")
    packr_d = nc.dram_tensor("packr", [P, PKR], F32R, kind="ExternalInput")
    biash_d = nc.dram_tensor("biash", [P, 1], F32, kind="ExternalInput")
    rowp_d = nc.dram_tensor("rowp", [1, P + QF], F32R, kind="ExternalInput")
    o_d = nc.dram_tensor("o", [P, KT, QF], F32, kind="ExternalOutput")

    with tile.TileContext(nc) as tc:
        with (
            tc.tile_pool(name="const", bufs=1) as cp,
            tc.tile_pool(name="persist", bufs=1) as pp,
            tc.tile_pool(name="wsp", bufs=3) as wsp,
            tc.tile_pool(name="psA", bufs=2, space="PSUM") as psA,
            tc.tile_pool(name="psSC", bufs=3, space="PSUM") as psSC,
            tc.tile_pool(name="psU", bufs=1, space="PSUM") as psU,
            tc.tile_pool(name="psN", bufs=1, space="PSUM") as psN,
        ):
            packr = cp.tile([P, PKR], F32R, tag="packr")
            wqk_t = cp.tile([P, KT, K], BF16, tag="wqk")
            wvo_t = packr[:, 0:o1].rearrange("p (t k) -> p t k", t=KT)
            bd_t = packr[:, o1:o3].rearrange("p (n t h) -> p n t h", n=NB, t=KT)
            bu_t = packr[:, o3:o4].rearrange("p (n t q) -> p n t q", n=NB, t=KT)
            onesc = packr[:, o4:o4 + 1]
            biash_t = cp.tile([P, 1], F32, tag="biash")
            rowp = cp.tile([1, P + QF], F32R, tag="rowp")
            onesr = rowp[:, 0:P]
            decq = rowp[:, P:P + QF]
            mask_t = cp.tile([P, DIAG, QF], BF16, tag="mask")
            eps1 = cp.tile([1, 1], F32, tag="eps1")
            wu_l = cp.tile([1, 1], BF16, tag="wul")
            wu_r = cp.tile([1, QF], BF16, tag="wur")

            gnT = pp.tile([P, KT, PW], BF16, tag="gnT")
            gnd = pp.tile([P, W, K], BF16, tag="gnd")
            gqT = pp.tile([P, KT, QF], F32, tag="gqT")
            qT = pp.tile([P, KT, QF], BF16, tag="qT")
            dec_bc = pp.tile([P, QF], F32R, tag="decbc")
            u_sb = pp.tile([P, KT, QF], F32R, tag="usb")
            g2T = pp.tile([P, KT, QF], F32, tag="g2T")
            gn2T = pp.tile([P, KT, QF], F32R, tag="gn2T")
            sq_t = pp.tile([P, KT, QF], F32R, tag="sq")
            h_sb = pp.tile([P, NB, QF], F32R, tag="h")
            rt_t = pp.tile([1, QF], F32, tag="rt")
            rr_t = pp.tile([1, QF], F32R, tag="rr")
            o_sb = pp.tile([P, KT, QF], F32, tag="osb")

            nc.vector.memset(eps1[:], EPS)
            nc.gpsimd.memset(wu_l[:], 0.0)
            nc.gpsimd.memset(wu_r[:], 0.0)
            # first Act op needs the Sqrt table so the preamble load picks
            # that set (keeps the load out of the post-attention chain)
            scr1 = cp.tile([1, 1], F32, tag="scr1")
            nc.scalar.activation(scr1[:], eps1[:], AF.Sqrt)

            # ---- DMAs, ordered by first use ----
            nc.sync.dma_start(wqk_t[:], wqk_d[:])
            nc.sync.dma_start(gnT[:, :, 0:QF], gnT_d[:, :, 0:QF])
            nc.sync.dma_start(mask_t[:], mask_d[:])
            nc.sync.dma_start(gnd[:], gnd_d[:])
            nc.sync.dma_start(rowp[:], rowp_d[:])
            if PW > QF:
                nc.sync.dma_start(gnT[:, :, QF:PW], gnT_d[:, :, QF:PW])
            nc.sync.dma_start(packr[:], packr_d[:])
            nc.sync.dma_start(gqT[:], gqT_d[:])
            nc.sync.dma_start(biash_t[:], biash_d[:])

            # ---- PE warm-up: keep PE busy from t~0 so the p-state ramp is
            # burned before the real matmuls ----
            for _ in range(NWU):
                wps = psA.tile([P, QF], F32, tag="mm")
                nc.tensor.matmul(wps[0:1, :], wu_l[:], wu_r[:],
                                 start=True, stop=True)

            # ---- q~ = (Wq Wk^T / sqrt(k))^T gn on the query columns ----
            for ko in range(KT):
                ps = psA.tile([P, QF], F32, tag="mm")
                for ki in range(KT):
                    nc.tensor.matmul(ps[:], wqk_t[:, ki, ko * P:(ko + 1) * P],
                                     gnT[:, ki, 0:QF],
                                     start=(ki == 0), stop=(ki == KT - 1))
                nc.vector.tensor_copy(qT[:, ko, :], ps[:])

            # decay^-(i+1) broadcast to all partitions
            psb = psA.tile([P, QF], F32, tag="mm")
            nc.tensor.matmul(psb[:], onesr, decq, start=True, stop=True)
            nc.scalar.copy(dec_bc[:], psb[:])

            # ---- windowed decayed attention ----
            u_ps = psU.tile([P, KT, QF], F32, tag="u")
            sc_list = []

            def finish(jt):
                w = wsp.tile([P, QF], BF16, tag="ws")
                if jt < DIAG:
                    nc.vector.tensor_mul(w[:], sc_list[jt][:], mask_t[:, jt, :])
                else:
                    nc.vector.tensor_copy(w[:], sc_list[jt][:])
                for kt in range(KT):
                    nc.tensor.matmul(u_ps[:, kt, :],
                                     gnd[:, jt, kt * P:(kt + 1) * P], w[:],
                                     start=(jt == 0), stop=(jt == W - 1))

            for jt in range(W):
                sc = psSC.tile([P, QF], F32, tag="sc")
                for ki in range(KT):
                    nc.tensor.matmul(sc[:], gnT[:, ki, jt * P:(jt + 1) * P],
                                     qT[:, ki, :],
                                     start=(ki == 0), stop=(ki == KT - 1))
                sc_list.append(sc)
                if jt >= 1:
                    finish(jt - 1)
            finish(W - 1)

            # ---- tail, software-pipelined in two half-width column
            # chunks, emitted stage-major (norms for both chunks, then MLP
            # for both) so Act sees Sqrt,Sqrt,Gelu,Gelu -- one table switch ----
            cs = psN.tile([1, QF], F32, tag="cs")
            up_ps = psU.tile([P, KT, QF], F32, tag="u")

            def tail_norm(cc):
                cl = slice(cc * HC, (cc + 1) * HC)
                # query-side decay factor applied once to u
                for kt in range(KT):
                    nc.vector.tensor_mul(u_sb[:, kt, cl], u_ps[:, kt, cl],
                                         dec_bc[:, cl])
                # mem = u @ (Wv Wo c); residual
                for ko in range(KT):
                    pst = psA.tile([P, QF], F32, tag="mm", name="pst")
                    ps = pst[:, 0:HC]
                    for ki in range(KT):
                        nc.tensor.matmul(ps, wvo_t[:, ki, ko * P:(ko + 1) * P],
                                         u_sb[:, ki, cl],
                                         start=(ki == 0), stop=(ki == KT - 1))
                    nc.vector.scalar_tensor_tensor(
                        g2T[:, ko, cl], ps, 1.0, gqT[:, ko, cl],
                        op0=OP.mult, op1=OP.add)
                # second rms-norm (k-major)
                for ki in range(KT):
                    nc.vector.tensor_mul(sq_t[:, ki, cl], g2T[:, ki, cl],
                                         g2T[:, ki, cl])
                for ki in range(KT):
                    nc.tensor.matmul(cs[:, cl], onesc, sq_t[:, ki, cl],
                                     start=(ki == 0), stop=(ki == KT - 1))
                nc.scalar.activation(rt_t[:, cl], cs[:, cl], AF.Sqrt,
                                     bias=eps1[:], scale=1.0 / K)
                with nc.allow_low_precision(reason="f32r is 32-bit storage"):
                    nc.vector.reciprocal(rr_t[:, cl], rt_t[:, cl])
                bct = psSC.tile([P, QF], F32, tag="sc", name="bct")
                bcp = bct[:, 0:HC]
                nc.tensor.matmul(bcp, onesr, rr_t[:, cl],
                                 start=True, stop=True)
                for ki in range(KT):
                    nc.vector.scalar_tensor_tensor(
                        gn2T[:, ki, cl], bcp, 1.0, g2T[:, ki, cl],
                        op0=OP.mult, op1=OP.mult)

            def tail_mlp(cc):
                cl = slice(cc * HC, (cc + 1) * HC)
                # dendritic MLP (gates folded into branch_up)
                for n in range(NB):
                    pst = psA.tile([P, QF], F32, tag="mm", name="pst")
                    ps = pst[:, 0:HC]
                    for ki in range(KT):
                        nc.tensor.matmul(ps, bd_t[:, n, ki, :],
                                         gn2T[:, ki, cl],
                                         start=(ki == 0), stop=(ki == KT - 1))
                    nc.scalar.activation(h_sb[:, n, cl], ps,
                                         AF.Gelu, bias=biash_t[:])
                for ko in range(KT):
                    for n in range(NB):
                        nc.tensor.matmul(up_ps[:, ko, cl], bu_t[:, n, ko, :],
                                         h_sb[:, n, cl],
                                         start=(n == 0), stop=(n == NB - 1))
                    nc.vector.tensor_copy(o_sb[:, ko, cl], up_ps[:, ko, cl])
                nc.sync.dma_start(o_d[:, :, cl], o_sb[:, :, cl])

            tail_norm(0)
            tail_norm(1)
            tail_mlp(0)
            tail_mlp(1)

    nc.compile()
    return nc


def _kernel_fast(g, gn, decay, Wq, Wk, Wv, Wo, c_mem, s_qk, branch_down,
                 bu_eff, mlp_bias, W):
    key = ("fast", W)
    nc = _prog_cache.get(key)
    if nc is None:
        nc = _build_fast(W)
        _prog_cache[key] = nc

    cw = W * P
    f64 = np.float64

    # Parameter folds (f64 host math, constants only)
    wqk = (Wq.astype(f64) @ Wk.astype(f64).T) * f64(s_qk)
    wqk_h = np.ascontiguousarray(
        wqk.reshape(KT, P, K).transpose(1, 0, 2)).astype(BFNP)
    wvo = (Wv.astype(f64) @ Wo.astype(f64)) * f64(c_mem)
    packr = np.concatenate([
        wvo.reshape(KT, P, K).transpose(1, 0, 2).reshape(P, -1),
        branch_down.reshape(NB, KT, P, INNER).transpose(2, 0, 1, 3).reshape(P, -1).astype(f64),
        bu_eff.transpose(1, 0, 2).reshape(P, -1),
        np.ones((P, 1), f64),
    ], axis=1).astype(np.float32)
    biash = mlp_bias.reshape(P, 1).astype(np.float32).copy()

    decq = np.power(f64(decay), -(np.arange(QF, dtype=f64) + 1.0))
    rowp = np.concatenate([np.ones(P, f64), decq]).reshape(1, -1).astype(np.float32)

    # Binary triangular masks for the diagonal j-tiles (core-independent)
    jj = np.arange(P)[:, None, None] + P * np.arange(DIAG)[None, :, None]
    mask = (jj > np.arange(QF)[None, None, :]).astype(BFNP)  # [P,DIAG,QF]

    dpow = np.power(f64(decay), np.arange(cw, dtype=f64)).astype(np.float32)

    in_maps = []
    for c in range(8):
        b, qc = divmod(c, NQ)
        base = qc * QF
        j_end = min(base + cw, T)
        nv = j_end - base
        gn_win = np.zeros((cw, K), np.float32)
        gn_win[:nv] = gn[b, base:j_end]
        gnT_h = np.ascontiguousarray(
            gn_win.T.reshape(KT, P, cw).transpose(1, 0, 2)).astype(BFNP)
        gnd_h = np.ascontiguousarray(
            (gn_win * dpow[:, None]).reshape(W, P, K).transpose(1, 0, 2)).astype(BFNP)
        gq = g[b, base:base + QF]
        gqT_h = np.ascontiguousarray(gq.T.reshape(KT, P, QF).transpose(1, 0, 2))
        in_maps.append({
            "wqk": wqk_h, "gnT": gnT_h, "gnd": gnd_h, "gqT": gqT_h,
            "mask": mask, "packr": packr, "biash": biash, "rowp": rowp,
        })

    res = run_bass_kernel_spmd(nc, in_maps, list(range(8)))
    return res


def kernel(**inputs):
    x = np.asarray(inputs["x"], np.float32)
    Wq = np.asarray(inputs["Wq"], np.float32)
    Wk = np.asarray(inputs["Wk"], np.float32)
    Wv = np.asarray(inputs["Wv"], np.float32)
    Wo = np.asarray(inputs["Wo"], np.float32)
    decay_logit = np.float32(np.asarray(inputs["decay_logit"]).reshape(()))
    out_scale = np.float32(np.asarray(inputs["out_scale"]).reshape(()))
    mem_scale = np.float32(np.asarray(inputs["mem_scale"]).reshape(-1)[0])
    branch_down = np.asarray(inputs["branch_down"], np.float32)
    branch_up = np.asarray(inputs["branch_up"], np.float32)
    mlp_bias = np.asarray(inputs["mlp_bias"], np.float32)
    gate_W = np.asarray(inputs["gate_W"], np.float32)
    gate_b = np.asarray(inputs["gate_b"], np.float32)
    write_scale = np.float32(np.asarray(inputs["write_scale"]).reshape(()))
    read_idx = np.asarray(inputs["read_indices"]).astype(np.int64)
    write_idx = np.asarray(inputs["write_indices"]).astype(np.int64)

    # Host-side gather of the active vocab subspace (data movement only).
    g = np.take(x, read_idx, axis=2)  # (B, T, K)

    decay = float(1.0 / (1.0 + np.exp(-np.float64(decay_logit))))
    s_qk = float(1.0 / np.sqrt(np.float32(K)))
    c_mem = float(out_scale * mem_scale)
    s_out = float(write_scale * np.float32(1.0 / 16.0))

    # Window size: j-tiles until the decay tail mass is < 1e-6
    if decay <= 0.0:
        ahead = 0
    elif decay >= 1.0 - 1e-9:
        ahead = NT
    else:
        d = math.log(1e-4 * (1.0 - decay)) / math.log(decay)
        ahead = max(0, min(NT, int(math.ceil(d / P))))
    W = min(DIAG + ahead, NT)

    decq_max = decay ** (-QF) if decay > 0 else float("inf")
    fast_ok = (not np.any(gate_W)) and np.isfinite(decq_max) and decq_max < 1e36

    if fast_ok:
        gb = gate_b.astype(np.float64)
        e = np.exp(gb - gb.max())
        gconst = e / e.sum()                       # softmax(gate_b), exact
        bu_eff = branch_up.astype(np.float64) * gconst[:, None, None] * s_out
        g64 = g.astype(np.float64)
        rinv = 1.0 / np.sqrt((g64 * g64).mean(axis=-1, keepdims=True) + EPS)
        gn = (g64 * rinv).astype(np.float32)
        res = _kernel_fast(g, gn, decay, Wq, Wk, Wv, Wo, c_mem, s_qk,
                           branch_down, bu_eff, mlp_bias, W)
        out = np.zeros((B, T, V), np.float32)
        for c in range(8):
            b, qc = divmod(c, NQ)
            oc = np.asarray(res.results[c]["o"], np.float32)  # [P, KT, QF]
            out[b, qc * QF:(qc + 1) * QF][:, write_idx] = (
                oc.transpose(1, 0, 2).reshape(K, QF).T)
        return out

    # ---------------- generic fallback (original program) ----------------
    return _kernel_generic(g, Wq, Wk, Wv, Wo, decay, c_mem, s_qk, s_out,
                           branch_down, branch_up, mlp_bias, gate_W, gate_b,
                           write_idx)


# ======================================================================
# Generic fallback: the original (unwindowed, on-device-gates) program.
# ======================================================================

def _build_generic(s_qk, c_mem, s_out):
    """Build the SPMD Bass/Tile program. Scalars are baked as immediates."""
    nc = bacc.Bacc("TRN2", target_bir_lowering=False, debug=False, num_devices=8)

    gT_d = nc.dram_tensor("gT", [KT, P, T], F32, kind="ExternalInput")
    gqT_d = nc.dram_tensor("gqT", [KT, P, QF], F32, kind="ExternalInput")
    wd2_d = nc.dram_tensor("wd", [NT // 2, P, 2, QF], F32, kind="ExternalInput")
    PK = 4 * KT * K + NB * KT * INNER + NB * K + KT * NB + 1
    pack_d = nc.dram_tensor("pack", [P, PK], F32R, kind="ExternalInput")
    onesc_d = nc.dram_tensor("onesc", [P, 1], F32R, kind="ExternalInput")
    biash_d = nc.dram_tensor("biash", [P, 1], F32, kind="ExternalInput")
    gateb_d = nc.dram_tensor("gateb", [P, NB], F32, kind="ExternalInput")
    onesr_d = nc.dram_tensor("onesr", [1, P], F32R, kind="ExternalInput")
    o_d = nc.dram_tensor("o", [NQ, P, K], F32, kind="ExternalOutput")

    WQ, WK, WV, WO = 0, 1, 2, 3
    AX = mybir.AxisListType.X

    with tile.TileContext(nc) as tc:
        with (
            tc.tile_pool(name="const", bufs=1) as cp,
            tc.tile_pool(name="persist", bufs=1) as pp,
            tc.tile_pool(name="work", bufs=3) as wp,
            tc.tile_pool(name="stat", bufs=4) as sp,
            tc.tile_pool(name="psM", bufs=4, space="PSUM") as psM,
            tc.tile_pool(name="psN", bufs=1, space="PSUM") as psN,
            tc.tile_pool(name="psR", bufs=1, space="PSUM") as psR,
        ):
            pack_t = cp.tile([P, PK], F32R, tag="pack")
            o1 = 4 * KT * K
            o2 = o1 + NB * KT * INNER
            o3 = o2 + NB * K
            o4 = o3 + KT * NB
            w_t = pack_t[:, 0:o1].rearrange("p (w t k) -> p w t k", w=4, t=KT)
            bd_t = pack_t[:, o1:o2].rearrange("p (n t h) -> p n t h", n=NB, t=KT)
            bu_t = pack_t[:, o2:o3].rearrange("p (n k) -> p n k", n=NB)
            gw_t = pack_t[:, o3:o4].rearrange("p (t n) -> p t n", t=KT)
            ones_col = cp.tile([P, 1], F32R, tag="ones_col")
            biash_t = cp.tile([P, 1], F32, tag="biash")
            gateb_t = cp.tile([P, NB], F32, tag="gateb")
            eps1_t = cp.tile([1, 1], F32, tag="eps1")
            nc.vector.memset(eps1_t[:], EPS)
            ones_row = cp.tile([1, P], F32R, tag="ones_row")

            gT = [pp.tile([P, T], F32, tag=f"gT{i}", name=f"gT{i}") for i in range(KT)]
            gqT = [pp.tile([P, QF], F32, tag=f"gqT{i}", name=f"gqT{i}") for i in range(KT)]
            gnT = [pp.tile([P, T], F32R, tag=f"gnT{i}", name=f"gnT{i}") for i in range(KT)]
            gqnT = [pp.tile([P, QF], F32R, tag=f"gqnT{i}", name=f"gqnT{i}") for i in range(KT)]
            kkT = [pp.tile([P, T], F32R, tag=f"kkT{i}", name=f"kkT{i}") for i in range(KT)]
            vv = [pp.tile([P, K], F32R, tag=f"vv{i}", name=f"vv{i}") for i in range(NT)]
            qT = [pp.tile([P, QF], F32R, tag=f"qT{i}", name=f"qT{i}") for i in range(KT)]
            retr_sb = [pp.tile([P, QF], F32R, tag=f"retr{i}", name=f"retr{i}") for i in range(KT)]
            g2T = [pp.tile([P, QF], F32, tag=f"g2T{i}", name=f"g2T{i}") for i in range(KT)]
            gn2T = [pp.tile([P, QF], F32R, tag=f"gn2T{i}", name=f"gn2T{i}") for i in range(KT)]
            h_sb = [pp.tile([P, QF], F32R, tag=f"h{n}", name=f"h{n}") for n in range(NB)]
            gates = [pp.tile([P, NB], F32, tag=f"gates{i}", name=f"gates{i}") for i in range(NQ)]
            o_sb = [pp.tile([P, K], F32, tag=f"o{i}", name=f"o{i}") for i in range(NQ)]

            def rms_norm_T(src, dst, cols, w):
                sq = wp.tile([P, KT, 512], F32R, tag="sq")
                for ki in range(KT):
                    nc.vector.tensor_mul(sq[:, ki, :w], src[ki][:, cols], src[ki][:, cols])
                cs = psN.tile([1, 512], F32, tag="cs")
                for ki in range(KT):
                    nc.tensor.matmul(cs[:1, :w], ones_col[:], sq[:, ki, :w],
                                     start=(ki == 0), stop=(ki == KT - 1))
                rt = sp.tile([1, 512], F32R, tag="rt")
                nc.scalar.activation(rt[:1, :w], cs[:1, :w], AF.Sqrt,
                                     bias=eps1_t[:], scale=1.0 / K)
                bc = psN.tile([P, 512], F32, tag="bc")
                nc.tensor.matmul(bc[:, :w], ones_row[:], rt[:1, :w],
                                 start=True, stop=True)
                rinv = wp.tile([P, 512], F32, tag="rinv")
                nc.vector.reciprocal(rinv[:, :w], bc[:, :w])
                for ki in range(KT):
                    nc.vector.scalar_tensor_tensor(
                        dst[ki][:, cols], rinv[:, :w], 1.0, src[ki][:, cols],
                        op0=OP.mult, op1=OP.mult)

            nc.sync.dma_start(ones_col[:], onesc_d[:])
            nc.sync.dma_start(ones_row[:], onesr_d[:])
            for ki in range(KT):
                nc.sync.dma_start(gT[ki][:, 0:512], gT_d[ki, :, 0:512])
            for ki in range(KT):
                nc.sync.dma_start(gqT[ki][:], gqT_d[ki])
            for jc in range(1, NC5):
                for ki in range(KT):
                    nc.sync.dma_start(gT[ki][:, jc * 512:(jc + 1) * 512],
                                      gT_d[ki, :, jc * 512:(jc + 1) * 512])
            nc.sync.dma_start(pack_t[:], pack_d[:])
            nc.sync.dma_start(biash_t[:], biash_d[:])
            nc.sync.dma_start(gateb_t[:], gateb_d[:])
            wd2 = [wp.tile([P, 2, QF], F32, tag=f"wd2_{jp}", name=f"wd2_{jp}", bufs=1)
                   for jp in range(NT // 2)]
            for jp in range(NT // 2):
                nc.sync.dma_start(wd2[jp][:], wd2_d[jp])
            rms_norm_T(gT, gnT, slice(0, 512), 512)
            rms_norm_T(gqT, gqnT, slice(0, QF), QF)
            for jc in range(1, NC5):
                rms_norm_T(gT, gnT, slice(jc * 512, (jc + 1) * 512), 512)

            for jc in range(NC5):
                for ko in range(KT):
                    ps = psM.tile([P, 512], F32, tag="mm")
                    for ki in range(KT):
                        nc.tensor.matmul(
                            ps[:], (w_t[:, WK, ki, ko * P:(ko + 1) * P]),
                            (gnT[ki][:, jc * 512:(jc + 1) * 512]),
                            start=(ki == 0), stop=(ki == KT - 1))
                    nc.scalar.copy(kkT[ko][:, jc * 512:(jc + 1) * 512], ps[:])
                for jt in range(4 * jc, 4 * jc + 4):
                    ps = psM.tile([P, K], F32, tag="mm")
                    for ki in range(KT):
                        nc.tensor.matmul(
                            ps[:], (gnT[ki][:, jt * P:(jt + 1) * P]), (w_t[:, WV, ki, :]),
                            start=(ki == 0), stop=(ki == KT - 1))
                    nc.vector.tensor_copy(vv[jt][:], ps[:])
            for ko in range(KT):
                ps = psM.tile([P, QF], F32, tag="mm")
                for ki in range(KT):
                    nc.tensor.matmul(
                        ps[:], (w_t[:, WQ, ki, ko * P:(ko + 1) * P]), (gqnT[ki][:]),
                        start=(ki == 0), stop=(ki == KT - 1))
                nc.scalar.mul(qT[ko][:], ps[:], s_qk)

            retr_ps = [psR.tile([P, QF], F32, tag=f"rps{kt}", name=f"rps{kt}")
                       for kt in range(KT)]
            for jt in range(NT):
                sc = psM.tile([P, QF], F32, tag="mm", name="sc")
                for ki in range(KT):
                    nc.tensor.matmul(
                        sc[:], (kkT[ki][:, jt * P:(jt + 1) * P]), (qT[ki][:]),
                        start=(ki == 0), stop=(ki == KT - 1))
                ws = wp.tile([P, QF], F32R, tag="ws")
                nc.vector.tensor_mul(ws[:], sc[:], wd2[jt // 2][:, jt % 2, :])
                for kt in range(KT):
                    nc.tensor.matmul(
                        retr_ps[kt][:], (vv[jt][:, kt * P:(kt + 1) * P]), (ws[:]),
                        start=(jt == 0), stop=(jt == NT - 1))
            for kt in range(KT):
                nc.vector.tensor_copy(retr_sb[kt][:], retr_ps[kt][:])

            for ko in range(KT):
                ps = psM.tile([P, QF], F32, tag="mm")
                for ki in range(KT):
                    nc.tensor.matmul(
                        ps[:], (w_t[:, WO, ki, ko * P:(ko + 1) * P]), (retr_sb[ki][:]),
                        start=(ki == 0), stop=(ki == KT - 1))
                nc.vector.scalar_tensor_tensor(
                    g2T[ko][:], ps[:], c_mem, gqT[ko][:],
                    op0=OP.mult, op1=OP.add)
            rms_norm_T(g2T, gn2T, slice(0, QF), QF)

            for n in range(NB):
                hp = psM.tile([P, QF], F32, tag="mm")
                for ki in range(KT):
                    nc.tensor.matmul(
                        hp[:], (bd_t[:, n, ki, :]), (gn2T[ki][:]),
                        start=(ki == 0), stop=(ki == KT - 1))
                nc.scalar.activation(h_sb[n][:], hp[:], AF.Gelu, bias=biash_t[:])

            for qt in range(NQ):
                gp = psM.tile([P, NB], F32, tag="mm")
                for ki in range(KT):
                    nc.tensor.matmul(
                        gp[:], gn2T[ki][:, qt * P:(qt + 1) * P], gw_t[:, ki, :],
                        start=(ki == 0), stop=(ki == KT - 1))
                gsb = sp.tile([P, NB], F32, tag="gsb")
                nc.vector.tensor_add(gsb[:], gp[:], gateb_t[:])
                mx = sp.tile([P, 1], F32, tag="mx")
                nc.vector.reduce_max(mx[:], gsb[:], axis=AX)
                sh = sp.tile([P, NB], F32, tag="sh")
                nc.vector.tensor_scalar(sh[:], gsb[:], mx[:], None, op0=OP.subtract)
                ex = sp.tile([P, NB], F32, tag="ex")
                nc.scalar.activation(ex[:], sh[:], AF.Exp)
                sm = sp.tile([P, 1], F32, tag="sm")
                nc.vector.reduce_sum(sm[:], ex[:], axis=AX)
                rc = sp.tile([P, 1], F32, tag="rc")
                nc.vector.reciprocal(rc[:], sm[:])
                nc.vector.tensor_scalar(
                    gates[qt][:], ex[:], rc[:], s_out, op0=OP.mult, op1=OP.mult)

            for qt in range(NQ):
                for n in range(NB):
                    bp = psM.tile([P, K], F32, tag="mm")
                    nc.tensor.matmul(
                        bp[:], (h_sb[n][:, qt * P:(qt + 1) * P]), (bu_t[:, n, :]),
                        start=True, stop=True)
                    if n == 0:
                        nc.vector.tensor_scalar_mul(o_sb[qt][:], bp[:], gates[qt][:, 0:1])
                    else:
                        nc.vector.scalar_tensor_tensor(
                            o_sb[qt][:], bp[:], gates[qt][:, n:n + 1], o_sb[qt][:],
                            op0=OP.mult, op1=OP.add)
                nc.sync.dma_start(o_d[qt], o_sb[qt][:])

    nc.compile()
    return nc


def _kernel_generic(g, Wq, Wk, Wv, Wo, decay, c_mem, s_qk, s_out,
                    branch_down, branch_up, mlp_bias, gate_W, gate_b,
                    write_idx):
    key = ("generic", round(s_qk, 12), round(c_mem, 12), round(s_out, 12))
    nc = _prog_cache.get(key)
    if nc is None:
        nc = _build_generic(s_qk, c_mem, s_out)
        _prog_cache[key] = nc

    wall = np.stack([Wq, Wk, Wv, Wo]).reshape(4, KT, P, K).transpose(2, 0, 1, 3)
    bdall = branch_down.reshape(NB, KT, P, INNER).transpose(2, 0, 1, 3)
    buall = branch_up.transpose(1, 0, 2)
    gw = gate_W.reshape(KT, P, NB).transpose(1, 0, 2)
    pack = np.concatenate([
        wall.reshape(P, -1), bdall.reshape(P, -1), buall.reshape(P, -1),
        gw.reshape(P, -1), np.ones((P, 1), np.float32)], axis=1).astype(np.float32)
    biash = mlp_bias.reshape(P, 1).copy()
    gateb = np.broadcast_to(gate_b, (P, NB)).copy()

    decay32 = np.float32(decay)
    jj = np.arange(T, dtype=np.float32)[:, None]
    gT_host = [np.ascontiguousarray(g[b].T).reshape(KT, P, T) for b in range(B)]
    in_maps = []
    for c in range(8):
        b, qc = divmod(c, NQ)
        ii = (np.arange(QF, dtype=np.float32) + qc * QF)[None, :]
        expo = np.maximum(jj - ii - np.float32(1.0), np.float32(0.0)).astype(np.float32)
        wdm = np.power(decay32, expo, dtype=np.float32)
        wdm[jj <= ii] = np.float32(0.0)
        gqT_host = np.ascontiguousarray(g[b][qc * QF:(qc + 1) * QF].T).reshape(KT, P, QF)
        in_maps.append({
            "gT": gT_host[b],
            "gqT": gqT_host,
            "wd": np.ascontiguousarray(wdm.reshape(NT // 2, 2, P, QF).swapaxes(1, 2)),
            "pack": pack, "biash": biash, "gateb": gateb,
            "onesc": np.ones((P, 1), np.float32),
            "onesr": np.ones((1, P), np.float32),
        })

    res = run_bass_kernel_spmd(nc, in_maps, list(range(8)))

    out = np.zeros((B, T, V), np.float32)
    for c in range(8):
        b, qc = divmod(c, NQ)
        oc = res.results[c]["o"].reshape(QF, K)
        out[b, qc * QF:(qc + 1) * QF, :][:, write_idx] = oc
    return out
